# revision 2
# baseline (speedup 1.0000x reference)
"""Trainium2 Bass kernel for EnhancedTransformerBlock (B=2,T=2048,C=1024,H=16,
SwiGLU HIDDEN=2730, ALiBi-abs + causal attention).

Sharding over 8 cores: batch (2) x head-groups (4 heads/core). Attention is
computed head-parallel with transposed scores S^T[tk,tq] so ALiBi + max-shift
collapse into a per-partition exp bias; softmax denominator rides as a ones
column in V through the PV matmul. A ReduceScatter over each batch group
converts the proj partials to row-sharded activations; the SwiGLU MLP then
runs row-parallel with full weights. Host reassembles row blocks.
"""
import sys, types
sys.path.insert(0, "/opt/trn_rl_repo")
import numpy as np
import ml_dtypes

import concourse.bass as bass
import concourse.tile as tile
from concourse import mybir
import concourse.bass_utils as bass_utils
import bass_rust
from concourse.masks import make_identity

# ----------------------------------------------------------------------------
# environment patches (walrus in this container accepts only 1 sync-wait/inst)
# ----------------------------------------------------------------------------
_DRAIN_WAIT_LIMIT = 1

def _patched_drain_and_barrier(self, tick_clock, wait_clock):
    nc = self.nc
    drain_inst = nc.sync.drain()
    wait_clock.add_sem_waits(
        drain_inst.ins, bass_rust.ScopedClock({None: tick_clock.global_clock})
    )
    si = drain_inst.ins.sync_info
    waits = list(si.on_wait) if si is not None else []
    if len(waits) > _DRAIN_WAIT_LIMIT:
        si.on_wait = waits[:_DRAIN_WAIT_LIMIT]
        for i in range(_DRAIN_WAIT_LIMIT, len(waits), _DRAIN_WAIT_LIMIT):
            d2 = nc.sync.drain()
            d2.ins.sync_info = bass_rust.SyncInfo(
                on_wait=waits[i:i + _DRAIN_WAIT_LIMIT], on_update=[]
            )
    nc.all_engine_barrier()
    popped = nc._tile_sem_poison_stack.pop()
    assert popped is self._sem_poison
    nc.clear_and_free_semaphores(list(self.sems.allocated().values()))
    nc.all_engine_barrier()


def _split_excess_waits(nc, limit=_DRAIN_WAIT_LIMIT):
    n = [0]
    for bb in nc.main_func.blocks:
        insts = bb.instructions
        out = []
        changed = False
        for inst in insts:
            si = inst.sync_info
            waits = list(si.on_wait) if si is not None else []
            if len(waits) > limit:
                changed = True
                keep = waits[-limit:]
                rest = waits[:-limit]
                for i in range(0, len(rest), limit):
                    n[0] += 1
                    d = mybir.InstNoOp(
                        name=f"waitsplit-{n[0]}", engine=inst.engine, ins=[], outs=[]
                    )
                    d.sync_info = bass_rust.SyncInfo(
                        on_wait=rest[i:i + limit], on_update=[]
                    )
                    out.append(d)
                si.on_wait = keep
            out.append(inst)
        if changed:
            bb.instructions = out


def _install_patches():
    tile.TileContext._drain_and_barrier = _patched_drain_and_barrier
    if "antenv.axon_hooks" not in sys.modules:
        try:
            from trn_agent_boot.trn_boot import _ntff_profile_via_ctypes
            hook = _ntff_profile_via_ctypes("/opt/axon/libaxon_pjrt.so")
        except Exception:
            hook = None
        mod = types.ModuleType("antenv.axon_hooks")
        mod.get_axon_ntff_profile_hook = lambda: hook
        mod.set_axon_ntff_profile_hook = lambda h: None
        sys.modules["antenv.axon_hooks"] = mod
        bass_utils.upload_artifacts = lambda tmpdir: tmpdir

_install_patches()

# ----------------------------------------------------------------------------
# problem constants
# ----------------------------------------------------------------------------
B, T, C = 2, 2048, 1024
H, D = 16, 64
HID, HIDP = 2730, 2816
N_CORES, TP = 8, 4
ROWS = T // TP          # 512 rows per core after reduce-scatter
HL = 4                  # local heads per core
EPS = 1e-5
CBIAS = 24.0
NEG = -1e30
F32, BF16 = mybir.dt.float32, mybir.dt.bfloat16
bf16 = ml_dtypes.bfloat16
ts = bass.ts

NTT = T // 128          # 16 token tiles
NTB = T // 512          # 4 token blocks
CCH = C // 128          # 8 feature chunks
MCH = HIDP // 128       # 22 hidden chunks
RT = ROWS // 128        # 4 row tiles per core


def _slopes():
    i = np.arange(1, H + 1, dtype=np.float64)
    return (1.0 / np.power(2.0, 8.0 * i / H)).astype(np.float64)


# ----------------------------------------------------------------------------
# device program (identical on all 8 cores; per-core behavior via input data)
# ----------------------------------------------------------------------------
def _build(nqkv):
    nc = bass.Bass("TRN2", num_devices=N_CORES)

    x_d = nc.dram_tensor("x_full", [T, C], F32, kind="ExternalInput")
    xr_d = nc.dram_tensor("x_rows", [ROWS, C], F32, kind="ExternalInput")
    wq_d = nc.dram_tensor("wq", [128, 2, 9, 128], BF16, kind="ExternalInput")
    wk_d = nc.dram_tensor("wk", [128, 2, 9, 128], BF16, kind="ExternalInput")
    wv_d = nc.dram_tensor("wv", [128, 9, 256], BF16, kind="ExternalInput")
    wp_d = nc.dram_tensor("wproj", [128, 2, 1024], BF16, kind="ExternalInput")
    wg_d = nc.dram_tensor("wg", [128, 8, HIDP], BF16, kind="ExternalInput")
    wu_d = nc.dram_tensor("wu", [128, 8, HIDP], BF16, kind="ExternalInput")
    wd_d = nc.dram_tensor("wd", [128, MCH, 1024], BF16, kind="ExternalInput")
    bg_d = nc.dram_tensor("bg", [128, MCH], F32, kind="ExternalInput")
    bu_d = nc.dram_tensor("bu", [128, MCH], F32, kind="ExternalInput")
    db_d = nc.dram_tensor("dbias", [128, 1024], F32, kind="ExternalInput")
    mk_d = nc.dram_tensor("masks", [128, 4, 512], F32, kind="ExternalInput")
    eb_d = nc.dram_tensor("ebias", [128, HL * 16], F32, kind="ExternalInput")

    out_d = nc.dram_tensor("out_rows", [ROWS, C], F32, kind="ExternalOutput")
    scr_d = nc.dram_tensor("scr", [16, 512], F32, kind="ExternalOutput")

    from contextlib import ExitStack
    with tile.TileContext(nc) as tc, ExitStack() as top:
        consts = top.enter_context(tc.tile_pool(name="consts", bufs=1))
        stats = top.enter_context(tc.tile_pool(name="stats", bufs=16))
        work = top.enter_context(tc.tile_pool(name="work", bufs=2))
        dramp = top.enter_context(tc.tile_pool(name="dram", bufs=1, space="DRAM"))

        ident = consts.tile([128, 128], BF16)
        make_identity(nc, ident[:])
        eps_ap = consts.tile([128, 1], F32)
        nc.vector.memset(eps_ap[:], EPS)
        masks = consts.tile([128, 4, 512], F32)
        nc.sync.dma_start(masks[:], mk_d[:, :, :])
        ebias = consts.tile([128, HL * 16], F32)
        nc.sync.dma_start(ebias[:], eb_d[:, :])
        bg_sb = consts.tile([128, MCH], F32)
        nc.sync.dma_start(bg_sb[:], bg_d[:, :])
        bu_sb = consts.tile([128, MCH], F32)
        nc.sync.dma_start(bu_sb[:], bu_d[:, :])
        db_sb = consts.tile([128, 1024], F32)
        nc.sync.dma_start(db_sb[:], db_d[:, :])

        rs_in = dramp.tile([T, C], F32)
        rs_out = dramp.tile([ROWS, C], F32)

        proj_scope = ExitStack()
        OnT_pool = proj_scope.enter_context(tc.tile_pool(name="OnT", bufs=1))
        OnT = OnT_pool.tile([128, 2, T], BF16)

        with ExitStack() as attn_scope:
            apool = attn_scope.enter_context(tc.tile_pool(name="attn", bufs=1))
            psA = attn_scope.enter_context(
                tc.tile_pool(name="psA", bufs=2, space="PSUM"))

            wq_sb = apool.tile([128, 2, 9, 128], BF16)
            nc.sync.dma_start(wq_sb[:], wq_d[:, :, :, :])
            wk_sb = apool.tile([128, 2, 9, 128], BF16)
            nc.sync.dma_start(wk_sb[:], wk_d[:, :, :, :])
            wv_sb = apool.tile([128, 9, 256], BF16)
            nc.sync.dma_start(wv_sb[:], wv_d[:, :, :])

            yT = apool.tile([128, nqkv, T], BF16)
            qT = apool.tile([128, HL, T], BF16)
            kT = apool.tile([128, HL, T], BF16)
            Vh = apool.tile([128, NTT, HL, 66], BF16)

            nc.vector.memset(qT[:], 0.0)
            nc.vector.memset(kT[:], 0.0)
            nc.scalar.memzero(Vh[:, :, :, 64:66])
            nc.vector.memset(Vh[:, :, :, 64:65], 1.0)
            if nqkv == 9:
                nc.vector.memset(yT[:, 8, :], 0.0)
                nc.vector.memset(yT[0:1, 8, :], 1.0)

            # ---- LN1 + transpose to feature-major yT --------------------
            for t in range(NTT):
                xt = work.tile([128, C], F32, tag="xt")
                nc.sync.dma_start(xt[:], x_d[ts(t, 128), :])
                ssum = stats.tile([128, 1], F32, tag="ssum")
                nc.vector.reduce_sum(ssum[:], xt[:], axis=mybir.AxisListType.X)
                sq = work.tile([128, C], F32, tag="sq")
                ssq = stats.tile([128, 1], F32, tag="ssq")
                nc.scalar.activation(sq[:], xt[:],
                                     mybir.ActivationFunctionType.Square,
                                     accum_out=ssq[:])
                mu = stats.tile([128, 1], F32, tag="mu")
                nc.vector.tensor_scalar_mul(mu[:], ssum[:], 1.0 / C)
                ssqn = stats.tile([128, 1], F32, tag="ssqn")
                nc.vector.tensor_scalar_mul(ssqn[:], ssq[:], 1.0 / C)
                musq = stats.tile([128, 1], F32, tag="musq")
                nc.vector.tensor_tensor(musq[:], mu[:], mu[:],
                                        mybir.AluOpType.mult)
                var = stats.tile([128, 1], F32, tag="var")
                nc.vector.tensor_tensor(var[:], ssqn[:], musq[:],
                                        mybir.AluOpType.subtract)
                std = stats.tile([128, 1], F32, tag="std")
                nc.scalar.activation(std[:], var[:],
                                     mybir.ActivationFunctionType.Sqrt,
                                     bias=eps_ap[:])
                rstd = stats.tile([128, 1], F32, tag="rstd")
                nc.vector.reciprocal(rstd[:], std[:])
                yb = work.tile([128, C], BF16, tag="yb")
                nc.vector.tensor_scalar(yb[:], xt[:], mu[:], rstd[:],
                                        mybir.AluOpType.subtract,
                                        mybir.AluOpType.mult)
                for half in range(2):
                    pt = psA.tile([128, 4, 128], BF16, tag="tr")
                    for i in range(4):
                        cc = half * 4 + i
                        nc.tensor.transpose(pt[:, i, :], yb[:, ts(cc, 128)],
                                            ident[:])
                    nc.vector.tensor_copy(
                        yT[:, half * 4:(half + 1) * 4, ts(t, 128)], pt[:])

            # ---- QKV ----------------------------------------------------
            for p in range(2):          # head pairs -> q/k feature-major
                for blk in range(NTB):
                    for which, wsb, dstT in (("q", wq_sb, qT), ("k", wk_sb, kT)):
                        ps = psA.tile([128, 512], F32, tag="qkv")
                        for cc in range(nqkv):
                            nc.tensor.matmul(ps[:], wsb[:, p, cc, :],
                                             yT[:, cc, ts(blk, 512)],
                                             start=(cc == 0),
                                             stop=(cc == nqkv - 1))
                        nc.vector.tensor_copy(dstT[0:64, 2 * p, ts(blk, 512)],
                                              ps[0:64, :])
                        nc.vector.tensor_copy(dstT[0:64, 2 * p + 1, ts(blk, 512)],
                                              ps[64:128, :])
            for t in range(NTT):        # V token-major
                ps = psA.tile([128, 256], F32, tag="qkv")
                for cc in range(nqkv):
                    nc.tensor.matmul(ps[:], yT[:, cc, ts(t, 128)],
                                     wv_sb[:, cc, :],
                                     start=(cc == 0), stop=(cc == nqkv - 1))
                nc.scalar.copy(Vh[:, t, :, 0:64],
                               ps.rearrange("p (h d) -> p h d", d=64))

            # ---- attention (S^T scores, ones-column denominator) --------
            with ExitStack() as ps_scope:
                psB = ps_scope.enter_context(
                    tc.tile_pool(name="psB", bufs=2, space="PSUM"))
                for h in range(HL):
                    for c in range(NTB):
                        po = psB.tile([128, 512], F32, tag="o")
                        t_hi = 4 * c + 4
                        for t in range(t_hi):
                            s = psB.tile([128, 512], F32, tag="s")
                            nc.tensor.matmul(s[:], kT[:, h, ts(t, 128)],
                                             qT[:, h, ts(c, 512)],
                                             start=True, stop=True)
                            if t >= 4 * c:
                                nc.vector.tensor_tensor(
                                    s[:], s[:], masks[:, t - 4 * c, :],
                                    mybir.AluOpType.add)
                            pT = work.tile([128, 512], BF16, tag="pT")
                            idx = h * 16 + t
                            nc.scalar.activation(
                                pT[:], s[:], mybir.ActivationFunctionType.Exp,
                                bias=ebias[:, idx:idx + 1])
                            nc.tensor.matmul(po[0:66, :], Vh[:, t, h, 0:66],
                                             pT[:], start=(t == 0),
                                             stop=(t == t_hi - 1))
                        den = work.tile([1, 512], F32, tag="den")
                        nc.vector.tensor_copy(den[:], po[64:65, :])
                        rc1 = work.tile([1, 512], F32, tag="rc1")
                        nc.vector.reciprocal(rc1[:], den[:])
                        i = h * 4 + c
                        nc.sync.dma_start(scr_d[i:i + 1, :], rc1[0:1, :])
                        rec = work.tile([64, 512], F32, tag="rec")
                        nc.sync.dma_start(
                            rec[:], scr_d[i:i + 1, :].to_broadcast((64, 512)))
                        nc.vector.tensor_tensor(
                            OnT[(h % 2) * 64:(h % 2) * 64 + 64, h // 2,
                                ts(c, 512)],
                            po[0:64, :], rec[:], mybir.AluOpType.mult)

        # ---- proj partial -> rs_in ----------------------------------
        with proj_scope as ps_scope:
            psP = ps_scope.enter_context(
                tc.tile_pool(name="psP", bufs=2, space="PSUM"))
            wp_sb = OnT_pool.tile([128, 2, 1024], BF16)
            nc.sync.dma_start(wp_sb[:], wp_d[:, :, :])
            for tt in range(NTT):
                for nb in range(2):
                    ps = psP.tile([128, 512], F32, tag="proj")
                    for cc in range(2):
                        nc.tensor.matmul(ps[:], OnT[:, cc, ts(tt, 128)],
                                         wp_sb[:, cc, ts(nb, 512)],
                                         start=(cc == 0), stop=(cc == 1))
                    pj = work.tile([128, 512], F32, tag="pj")
                    nc.scalar.copy(pj[:], ps[:])
                    nc.sync.dma_start(rs_in[ts(tt, 128), ts(nb, 512)], pj[:])

        # ---- reduce-scatter over the 4-core batch group -------------
        nc.gpsimd.collective_compute(
            "ReduceScatter", mybir.AluOpType.add,
            replica_groups=[[0, 1, 2, 3], [4, 5, 6, 7]],
            ins=[rs_in.opt()], outs=[rs_out.opt()],
        )

        # ---- residual + LN2 + SwiGLU MLP (row-parallel) -------------
        with ExitStack() as mlp_scope:
            mpool = mlp_scope.enter_context(tc.tile_pool(name="mlp", bufs=1))
            wstream = mlp_scope.enter_context(tc.tile_pool(name="wstream", bufs=2))
            wdpool = mlp_scope.enter_context(tc.tile_pool(name="wdpool", bufs=1))
            psC = mlp_scope.enter_context(
                tc.tile_pool(name="psC", bufs=2, space="PSUM"))

            x2 = mpool.tile([128, RT, C], F32)
            y2T = mpool.tile([128, 8, ROWS], BF16)
            gu = mpool.tile([128, MCH, ROWS], BF16)

            for r in range(RT):
                rst = work.tile([128, C], F32, tag="rst")
                nc.sync.dma_start(rst[:], rs_out[ts(r, 128), :])
                xrt = work.tile([128, C], F32, tag="xrt")
                nc.sync.dma_start(xrt[:], xr_d[ts(r, 128), :])
                nc.vector.tensor_tensor(x2[:, r, :], rst[:], xrt[:],
                                        mybir.AluOpType.add)
                ssum = stats.tile([128, 1], F32, tag="ssum")
                nc.vector.reduce_sum(ssum[:], x2[:, r, :],
                                     axis=mybir.AxisListType.X)
                sq = work.tile([128, C], F32, tag="sq")
                ssq = stats.tile([128, 1], F32, tag="ssq")
                nc.scalar.activation(sq[:], x2[:, r, :],
                                     mybir.ActivationFunctionType.Square,
                                     accum_out=ssq[:])
                mu = stats.tile([128, 1], F32, tag="mu")
                nc.vector.tensor_scalar_mul(mu[:], ssum[:], 1.0 / C)
                ssqn = stats.tile([128, 1], F32, tag="ssqn")
                nc.vector.tensor_scalar_mul(ssqn[:], ssq[:], 1.0 / C)
                musq = stats.tile([128, 1], F32, tag="musq")
                nc.vector.tensor_tensor(musq[:], mu[:], mu[:],
                                        mybir.AluOpType.mult)
                var = stats.tile([128, 1], F32, tag="var")
                nc.vector.tensor_tensor(var[:], ssqn[:], musq[:],
                                        mybir.AluOpType.subtract)
                std = stats.tile([128, 1], F32, tag="std")
                nc.scalar.activation(std[:], var[:],
                                     mybir.ActivationFunctionType.Sqrt,
                                     bias=eps_ap[:])
                rstd = stats.tile([128, 1], F32, tag="rstd")
                nc.vector.reciprocal(rstd[:], std[:])
                yb = work.tile([128, C], BF16, tag="yb")
                nc.vector.tensor_scalar(yb[:], x2[:, r, :], mu[:], rstd[:],
                                        mybir.AluOpType.subtract,
                                        mybir.AluOpType.mult)
                for half in range(2):
                    pt = psC.tile([128, 4, 128], BF16, tag="tr2")
                    for i in range(4):
                        cc = half * 4 + i
                        nc.tensor.transpose(pt[:, i, :], yb[:, ts(cc, 128)],
                                            ident[:])
                    nc.vector.tensor_copy(
                        y2T[:, half * 4:(half + 1) * 4, ts(r, 128)], pt[:])

            for hc in range(MCH):
                wgt = wstream.tile([128, 8, 128], BF16, tag="wgt")
                nc.sync.dma_start(wgt[:], wg_d[:, :, ts(hc, 128)])
                wut = wstream.tile([128, 8, 128], BF16, tag="wut")
                nc.sync.dma_start(wut[:], wu_d[:, :, ts(hc, 128)])
                pg = psC.tile([128, 512], F32, tag="g")
                pu = psC.tile([128, 512], F32, tag="u")
                for cc in range(CCH):
                    nc.tensor.matmul(pg[:], wgt[:, cc, :], y2T[:, cc, :],
                                     start=(cc == 0), stop=(cc == CCH - 1))
                for cc in range(CCH):
                    nc.tensor.matmul(pu[:], wut[:, cc, :], y2T[:, cc, :],
                                     start=(cc == 0), stop=(cc == CCH - 1))
                gs = work.tile([128, 512], BF16, tag="gs")
                nc.scalar.activation(gs[:], pg[:],
                                     mybir.ActivationFunctionType.Silu,
                                     bias=bg_sb[:, hc:hc + 1])
                us = work.tile([128, 512], BF16, tag="us")
                nc.scalar.activation(us[:], pu[:],
                                     mybir.ActivationFunctionType.Identity,
                                     bias=bu_sb[:, hc:hc + 1])
                nc.vector.tensor_tensor(gu[:, hc, :], gs[:], us[:],
                                        mybir.AluOpType.mult)

            for nb in range(2):
                wdt = wdpool.tile([128, MCH, 512], BF16, tag="wdt")
                nc.sync.dma_start(wdt[:], wd_d[:, :, ts(nb, 512)])
                for tt in range(RT):
                    pd = psC.tile([128, 512], F32, tag="d")
                    for hc in range(MCH):
                        nc.tensor.matmul(pd[:], gu[:, hc, ts(tt, 128)],
                                         wdt[:, hc, :],
                                         start=(hc == 0), stop=(hc == MCH - 1))
                    o1 = work.tile([128, 512], F32, tag="o1")
                    nc.vector.tensor_tensor(o1[:], pd[:],
                                            x2[:, tt, ts(nb, 512)],
                                            mybir.AluOpType.add)
                    nc.vector.tensor_tensor(o1[:], o1[:],
                                            db_sb[:, ts(nb, 512)],
                                            mybir.AluOpType.add)
                    nc.sync.dma_start(out_d[ts(tt, 128), ts(nb, 512)], o1[:])

    _split_excess_waits(nc)
    return nc


# ----------------------------------------------------------------------------
# host-side input prep + launch
# ----------------------------------------------------------------------------
_cache = {}

def _get_nc(nqkv):
    if nqkv not in _cache:
        _cache[nqkv] = _build(nqkv)
    return _cache[nqkv]


def kernel(x, ln1_g, ln1_b, qkv_w, qkv_b, proj_w, proj_b,
           ln2_g, ln2_b, gate_w, gate_b, up_w, up_b, down_w, down_b):
    x = np.asarray(x, np.float32)
    f = lambda a: np.asarray(a, np.float32)
    ln1_g, ln1_b, qkv_b, proj_b, ln2_g, ln2_b = map(f, (
        ln1_g, ln1_b, qkv_b, proj_b, ln2_g, ln2_b))
    qkv_w, proj_w, gate_w, gate_b, up_w, up_b, down_w, down_b = map(f, (
        qkv_w, proj_w, gate_w, gate_b, up_w, up_b, down_w, down_b))

    slopes = _slopes()

    # fold LN affines into the consuming matmuls
    w1 = qkv_w * ln1_g[:, None]
    b1 = ln1_b @ qkv_w + qkv_b              # [3C]
    wg_f = gate_w * ln2_g[:, None]
    bg_f = ln2_b @ gate_w + gate_b          # [HID]
    wu_f = up_w * ln2_g[:, None]
    bu_f = ln2_b @ up_w + up_b

    nqkv = 9 if np.any(b1 != 0.0) else 8

    def qkv_aug(wcols, bcols, scale):
        # [C, 256] + bias row -> device layout [128, 2, 9, 128]
        wa = np.zeros((1152, 256), np.float32)
        wa[:C] = wcols * scale
        wa[C] = bcols * scale
        dev = np.zeros((128, 2, 9, 128), bf16)
        for p in range(2):
            blk = wa[:, p * 128:(p + 1) * 128]            # [1152, 128]
            dev[:, p] = blk.reshape(9, 128, 128).transpose(1, 0, 2).astype(bf16)
        return dev

    # masks (S^T diagonal tiles) and per-head exp biases
    pp = np.arange(128)[:, None]
    jj = np.arange(512)[None, :]
    masks_np = np.zeros((128, 4, 512), np.float32)
    for v in range(4):
        masks_np[:, v, :] = np.where(jj >= pp + 128 * v, 0.0, NEG)

    wgp = np.zeros((C, HIDP), np.float32); wgp[:, :HID] = wg_f
    wup = np.zeros((C, HIDP), np.float32); wup[:, :HID] = wu_f
    wdp = np.zeros((HIDP, 1024), np.float32); wdp[:HID] = down_w
    bgp = np.zeros(HIDP, np.float32); bgp[:HID] = bg_f
    bup = np.zeros(HIDP, np.float32); bup[:HID] = bu_f

    wg_dev = wgp.reshape(8, 128, HIDP).transpose(1, 0, 2).astype(bf16)
    wu_dev = wup.reshape(8, 128, HIDP).transpose(1, 0, 2).astype(bf16)
    wd_dev = wdp.reshape(MCH, 128, 1024).transpose(1, 0, 2).astype(bf16)
    bg_dev = bgp.reshape(MCH, 128).T.copy()
    bu_dev = bup.reshape(MCH, 128).T.copy()
    db_dev = np.broadcast_to(down_b, (128, 1024)).copy()

    in_maps = []
    for c in range(N_CORES):
        b, g = c // TP, c % TP
        heads = range(4 * g, 4 * g + 4)
        qcols = np.concatenate([np.arange(h * D, (h + 1) * D) for h in heads])
        kcols = qcols + C
        vcols = qcols + 2 * C

        wq_dev = qkv_aug(w1[:, qcols], b1[qcols], 0.125)
        wk_dev = qkv_aug(w1[:, kcols], b1[kcols], 1.0)
        wv_a = np.zeros((1152, 256), np.float32)
        wv_a[:C] = w1[:, vcols]
        wv_a[C] = b1[vcols]
        wv_dev = wv_a.reshape(9, 128, 256).transpose(1, 0, 2).astype(bf16)

        wp_rows = proj_w[qcols, :]                        # [256, 1024]
        wp_dev = wp_rows.reshape(2, 128, 1024).transpose(1, 0, 2).astype(bf16)

        eb = np.zeros((128, HL * 16), np.float32)
        for hl, h in enumerate(heads):
            sl = slopes[h]
            for t in range(16):
                eb[:, hl * 16 + t] = (-sl * (128 * t + np.arange(128))
                                      - CBIAS).astype(np.float32)

        in_maps.append({
            "x_full": x[b],
            "x_rows": x[b, ROWS * g:ROWS * (g + 1)] + proj_b[None, :],
            "wq": wq_dev, "wk": wk_dev, "wv": wv_dev, "wproj": wp_dev,
            "wg": wg_dev, "wu": wu_dev, "wd": wd_dev,
            "bg": bg_dev, "bu": bu_dev, "dbias": db_dev,
            "masks": masks_np, "ebias": eb,
        })

    nc = _get_nc(nqkv)
    res = bass_utils.run_bass_kernel_spmd(
        nc, in_maps, core_ids=list(range(N_CORES)))

    out = np.empty((B, T, C), np.float32)
    for c in range(N_CORES):
        b, g = c // TP, c % TP
        out[b, ROWS * g:ROWS * (g + 1)] = res.results[c]["out_rows"]
    return out


# revision 6
# speedup vs baseline: 1.2434x; 1.2434x over previous
"""Trainium2 Bass kernel for EnhancedTransformerBlock (B=2,T=2048,C=1024,H=16,
SwiGLU HIDDEN=2730, ALiBi-abs + causal attention).

Sharding over 8 cores: batch (2) x head-groups (4 heads/core). Attention is
computed head-parallel with transposed scores S^T[tk,tq] so ALiBi + max-shift
collapse into a per-partition exp bias; softmax denominator rides as a ones
column in V through the PV matmul. A ReduceScatter over each batch group
converts the proj partials to row-sharded activations; the SwiGLU MLP then
runs row-parallel with full weights. Host reassembles row blocks.
"""
import sys, types
sys.path.insert(0, "/opt/trn_rl_repo")
import numpy as np
import ml_dtypes

import concourse.bass as bass
import concourse.tile as tile
from concourse import mybir
import concourse.bass_utils as bass_utils
import bass_rust
from concourse.masks import make_identity

# ----------------------------------------------------------------------------
# environment patches (walrus in this container accepts only 1 sync-wait/inst)
# ----------------------------------------------------------------------------
_DRAIN_WAIT_LIMIT = 1

def _patched_drain_and_barrier(self, tick_clock, wait_clock):
    nc = self.nc
    drain_inst = nc.sync.drain()
    wait_clock.add_sem_waits(
        drain_inst.ins, bass_rust.ScopedClock({None: tick_clock.global_clock})
    )
    si = drain_inst.ins.sync_info
    waits = list(si.on_wait) if si is not None else []
    if len(waits) > _DRAIN_WAIT_LIMIT:
        si.on_wait = waits[:_DRAIN_WAIT_LIMIT]
        for i in range(_DRAIN_WAIT_LIMIT, len(waits), _DRAIN_WAIT_LIMIT):
            d2 = nc.sync.drain()
            d2.ins.sync_info = bass_rust.SyncInfo(
                on_wait=waits[i:i + _DRAIN_WAIT_LIMIT], on_update=[]
            )
    nc.all_engine_barrier()
    popped = nc._tile_sem_poison_stack.pop()
    assert popped is self._sem_poison
    nc.clear_and_free_semaphores(list(self.sems.allocated().values()))
    nc.all_engine_barrier()


def _split_excess_waits(nc, limit=_DRAIN_WAIT_LIMIT):
    n = [0]
    for bb in nc.main_func.blocks:
        insts = bb.instructions
        out = []
        changed = False
        for inst in insts:
            si = inst.sync_info
            waits = list(si.on_wait) if si is not None else []
            if len(waits) > limit:
                changed = True
                keep = waits[-limit:]
                rest = waits[:-limit]
                for i in range(0, len(rest), limit):
                    n[0] += 1
                    d = mybir.InstNoOp(
                        name=f"waitsplit-{n[0]}", engine=inst.engine, ins=[], outs=[]
                    )
                    d.sync_info = bass_rust.SyncInfo(
                        on_wait=rest[i:i + limit], on_update=[]
                    )
                    out.append(d)
                si.on_wait = keep
            out.append(inst)
        if changed:
            bb.instructions = out


def _install_patches():
    tile.TileContext._drain_and_barrier = _patched_drain_and_barrier
    if "antenv.axon_hooks" not in sys.modules:
        try:
            from trn_agent_boot.trn_boot import _ntff_profile_via_ctypes
            hook = _ntff_profile_via_ctypes("/opt/axon/libaxon_pjrt.so")
        except Exception:
            hook = None
        mod = types.ModuleType("antenv.axon_hooks")
        mod.get_axon_ntff_profile_hook = lambda: hook
        mod.set_axon_ntff_profile_hook = lambda h: None
        sys.modules["antenv.axon_hooks"] = mod
        bass_utils.upload_artifacts = lambda tmpdir: tmpdir

_install_patches()

# ----------------------------------------------------------------------------
# problem constants
# ----------------------------------------------------------------------------
B, T, C = 2, 2048, 1024
H, D = 16, 64
HID, HIDP = 2730, 2816
N_CORES, TP = 8, 4
ROWS = T // TP          # 512 rows per core after reduce-scatter
HL = 4                  # local heads per core
EPS = 1e-5
CBIAS = 24.0
NEG = -1e30
F32, BF16 = mybir.dt.float32, mybir.dt.bfloat16
bf16 = ml_dtypes.bfloat16
ts = bass.ts

NTT = T // 128          # 16 token tiles
NTB = T // 512          # 4 token blocks
CCH = C // 128          # 8 feature chunks
MCH = HIDP // 128       # 22 hidden chunks
RT = ROWS // 128        # 4 row tiles per core


def _slopes():
    i = np.arange(1, H + 1, dtype=np.float64)
    return (1.0 / np.power(2.0, 8.0 * i / H)).astype(np.float64)


# ----------------------------------------------------------------------------
# device program (identical on all 8 cores; per-core behavior via input data)
# ----------------------------------------------------------------------------
def _build(nqkv):
    nc = bass.Bass("TRN2", num_devices=N_CORES)

    x_d = nc.dram_tensor("x_full", [T, C], F32, kind="ExternalInput")
    xr_d = nc.dram_tensor("x_rows", [ROWS, C], F32, kind="ExternalInput")
    wq_d = nc.dram_tensor("wq", [128, 2, 9, 128], BF16, kind="ExternalInput")
    wk_d = nc.dram_tensor("wk", [128, 2, 9, 128], BF16, kind="ExternalInput")
    wv_d = nc.dram_tensor("wv", [128, 9, 256], BF16, kind="ExternalInput")
    wp_d = nc.dram_tensor("wproj", [128, 2, 1024], BF16, kind="ExternalInput")
    wg_d = nc.dram_tensor("wg", [128, 8, HIDP], BF16, kind="ExternalInput")
    wu_d = nc.dram_tensor("wu", [128, 8, HIDP], BF16, kind="ExternalInput")
    wd_d = nc.dram_tensor("wd", [128, MCH, 1024], BF16, kind="ExternalInput")
    bg_d = nc.dram_tensor("bg", [128, MCH], F32, kind="ExternalInput")
    bu_d = nc.dram_tensor("bu", [128, MCH], F32, kind="ExternalInput")
    db_d = nc.dram_tensor("dbias", [128, 1024], F32, kind="ExternalInput")
    mk_d = nc.dram_tensor("masks", [128, 4, 512], BF16, kind="ExternalInput")
    eb_d = nc.dram_tensor("ebias", [128, HL * 16], F32, kind="ExternalInput")

    out_d = nc.dram_tensor("out_rows", [ROWS, C], F32, kind="ExternalOutput")
    scr_d = nc.dram_tensor("scr", [16, 512], F32, kind="ExternalOutput")

    from contextlib import ExitStack
    with tile.TileContext(nc) as tc, ExitStack() as top:
        consts = top.enter_context(tc.tile_pool(name="consts", bufs=1))
        stats = top.enter_context(tc.tile_pool(name="stats", bufs=16))
        work = top.enter_context(tc.tile_pool(name="work", bufs=2))
        dramp = top.enter_context(tc.tile_pool(name="dram", bufs=1, space="DRAM"))

        ident = consts.tile([128, 128], BF16)
        make_identity(nc, ident[:])
        eps_ap = consts.tile([128, 1], F32)
        nc.vector.memset(eps_ap[:], EPS)
        masks = consts.tile([128, 4, 512], BF16)
        nc.sync.dma_start(masks[:], mk_d[:, :, :])
        ebias = consts.tile([128, HL * 16], F32)
        nc.sync.dma_start(ebias[:], eb_d[:, :])
        bg_sb = consts.tile([128, MCH], F32)
        nc.sync.dma_start(bg_sb[:], bg_d[:, :])
        bu_sb = consts.tile([128, MCH], F32)
        nc.sync.dma_start(bu_sb[:], bu_d[:, :])
        db_sb = consts.tile([128, 1024], F32)
        nc.sync.dma_start(db_sb[:], db_d[:, :])

        rs_ins = [dramp.tile([512, C], BF16, name=f"rsin{i}") for i in range(NTB)]
        rs_outs = [dramp.tile([128, C], BF16, name=f"rsout{i}") for i in range(NTB)]

        with ExitStack() as attn_scope:
            apool = attn_scope.enter_context(tc.tile_pool(name="attn", bufs=1))
            pipe = attn_scope.enter_context(tc.tile_pool(name="pipe", bufs=4))
            OnTp = attn_scope.enter_context(tc.tile_pool(name="OnTp", bufs=2))

            wq_sb = apool.tile([128, 2, 9, 128], BF16)
            nc.sync.dma_start(wq_sb[:], wq_d[:, :, :, :])
            wk_sb = apool.tile([128, 2, 9, 128], BF16)
            nc.sync.dma_start(wk_sb[:], wk_d[:, :, :, :])
            wv_sb = apool.tile([128, 9, 256], BF16)
            nc.sync.dma_start(wv_sb[:], wv_d[:, :, :])
            wp_sb = apool.tile([128, 2, 1024], BF16)
            nc.sync.dma_start(wp_sb[:], wp_d[:, :, :])

            yT = apool.tile([128, nqkv, T], BF16)
            qT = apool.tile([128, HL, T], BF16)
            kT = apool.tile([128, HL, T], BF16)
            Vh = apool.tile([128, NTT, HL, 66], BF16)

            nc.vector.memset(qT[:], 0.0)
            nc.vector.memset(kT[:], 0.0)
            nc.scalar.memzero(Vh[:, :, :, 64:66])
            nc.vector.memset(Vh[:, :, :, 64:65], 1.0)
            if nqkv == 9:
                nc.vector.memset(yT[:, 8, :], 0.0)
                nc.vector.memset(yT[0:1, 8, :], 1.0)

            with ExitStack() as qkv_scope:
                psA = qkv_scope.enter_context(
                    tc.tile_pool(name="psA", bufs=2, space="PSUM"))

                # ---- LN1 + transpose to feature-major yT ----------------
                for t in range(NTT):
                    xt = work.tile([128, C], F32, tag="xt")
                    nc.sync.dma_start(xt[:], x_d[ts(t, 128), :])
                    ssum = stats.tile([128, 1], F32, tag="ssum")
                    nc.vector.reduce_sum(ssum[:], xt[:],
                                         axis=mybir.AxisListType.X)
                    sq = work.tile([128, C], F32, tag="sq")
                    ssq = stats.tile([128, 1], F32, tag="ssq")
                    nc.scalar.activation(sq[:], xt[:],
                                         mybir.ActivationFunctionType.Square,
                                         accum_out=ssq[:])
                    mu = stats.tile([128, 1], F32, tag="mu")
                    nc.vector.tensor_scalar_mul(mu[:], ssum[:], 1.0 / C)
                    ssqn = stats.tile([128, 1], F32, tag="ssqn")
                    nc.vector.tensor_scalar_mul(ssqn[:], ssq[:], 1.0 / C)
                    musq = stats.tile([128, 1], F32, tag="musq")
                    nc.vector.tensor_tensor(musq[:], mu[:], mu[:],
                                            mybir.AluOpType.mult)
                    var = stats.tile([128, 1], F32, tag="var")
                    nc.vector.tensor_tensor(var[:], ssqn[:], musq[:],
                                            mybir.AluOpType.subtract)
                    std = stats.tile([128, 1], F32, tag="std")
                    nc.scalar.activation(std[:], var[:],
                                         mybir.ActivationFunctionType.Sqrt,
                                         bias=eps_ap[:])
                    rstd = stats.tile([128, 1], F32, tag="rstd")
                    nc.vector.reciprocal(rstd[:], std[:])
                    yb = work.tile([128, C], BF16, tag="yb")
                    nc.vector.tensor_scalar(yb[:], xt[:], mu[:], rstd[:],
                                            mybir.AluOpType.subtract,
                                            mybir.AluOpType.mult)
                    for half in range(2):
                        pt = psA.tile([128, 4, 128], BF16, tag="tr")
                        for i in range(4):
                            cc = half * 4 + i
                            nc.tensor.transpose(pt[:, i, :],
                                                yb[:, ts(cc, 128)], ident[:])
                        nc.vector.tensor_copy(
                            yT[:, half * 4:(half + 1) * 4, ts(t, 128)], pt[:])

                # ---- QKV ------------------------------------------------
                for p in range(2):      # head pairs -> q/k feature-major
                    for blk in range(NTB):
                        for wsb, dstT in ((wq_sb, qT), (wk_sb, kT)):
                            ps = psA.tile([128, 512], F32, tag="qkv")
                            for cc in range(nqkv):
                                nc.tensor.matmul(ps[:], wsb[:, p, cc, :],
                                                 yT[:, cc, ts(blk, 512)],
                                                 start=(cc == 0),
                                                 stop=(cc == nqkv - 1))
                            nc.vector.tensor_copy(
                                dstT[0:64, 2 * p, ts(blk, 512)], ps[0:64, :])
                            nc.vector.tensor_copy(
                                dstT[0:64, 2 * p + 1, ts(blk, 512)],
                                ps[64:128, :])
                for t in range(NTT):    # V token-major
                    ps = psA.tile([128, 256], F32, tag="qkv")
                    for cc in range(nqkv):
                        nc.tensor.matmul(ps[:], yT[:, cc, ts(t, 128)],
                                         wv_sb[:, cc, :],
                                         start=(cc == 0), stop=(cc == nqkv - 1))
                    nc.scalar.copy(Vh[:, t, :, 0:64],
                                   ps.rearrange("p (h d) -> p h d", d=64))

            # ---- attention + per-block proj + pipelined RS --------------
            with ExitStack() as ps_scope:
                psB = ps_scope.enter_context(
                    tc.tile_pool(name="psB", bufs=2, space="PSUM"))
                psS = ps_scope.enter_context(
                    tc.tile_pool(name="psS", bufs=3, space="PSUM"))
                for c in range(NTB):
                    OnT_c = OnTp.tile([128, 2, 512], BF16, tag="OnT")
                    for h in range(HL):
                        po = psB.tile([128, 512], F32, tag="o")
                        t_hi = 4 * c + 4
                        for t in range(t_hi):
                            s = psS.tile([128, 512], F32, tag="s")
                            nc.tensor.matmul(s[:], kT[:, h, ts(t, 128)],
                                             qT[:, h, ts(c, 512)],
                                             start=True, stop=True)
                            pT = pipe.tile([128, 512], BF16, tag="pT")
                            idx = h * 16 + t
                            nc.scalar.activation(
                                pT[:], s[:], mybir.ActivationFunctionType.Exp,
                                bias=ebias[:, idx:idx + 1])
                            if t >= 4 * c:
                                nc.vector.tensor_tensor(
                                    pT[:], pT[:], masks[:, t - 4 * c, :],
                                    mybir.AluOpType.mult)
                            nc.tensor.matmul(po[0:66, :], Vh[:, t, h, 0:66],
                                             pT[:], start=(t == 0),
                                             stop=(t == t_hi - 1))
                        i = h * 4 + c
                        den = pipe.tile([1, 512], F32, tag="den")
                        nc.vector.tensor_copy(den[:], po[64:65, :])
                        nc.sync.dma_start(scr_d[i:i + 1, :], den[0:1, :])
                        denb = pipe.tile([64, 512], F32, tag="denb")
                        nc.sync.dma_start(
                            denb[:], scr_d[i:i + 1, :].to_broadcast((64, 512)))
                        rec = pipe.tile([64, 512], F32, tag="rec")
                        nc.vector.reciprocal(rec[:], denb[:])
                        nc.vector.tensor_tensor(
                            OnT_c[(h % 2) * 64:(h % 2) * 64 + 64, h // 2, :],
                            po[0:64, :], rec[:], mybir.AluOpType.mult)
                    # proj partial for this token block -> reduce-scatter
                    for i2 in range(4):
                        for nb in range(2):
                            pp_ = psB.tile([128, 512], F32, tag="pj")
                            for cc in range(2):
                                nc.tensor.matmul(pp_[:],
                                                 OnT_c[:, cc, ts(i2, 128)],
                                                 wp_sb[:, cc, ts(nb, 512)],
                                                 start=(cc == 0), stop=(cc == 1))
                            pjs = pipe.tile([128, 512], BF16, tag="pjs")
                            nc.scalar.copy(pjs[:], pp_[:])
                            nc.sync.dma_start(
                                rs_ins[c][ts(i2, 128), ts(nb, 512)], pjs[:])
                    nc.gpsimd.collective_compute(
                        "ReduceScatter", mybir.AluOpType.add,
                        replica_groups=[[0, 1, 2, 3], [4, 5, 6, 7]],
                        ins=[rs_ins[c].opt()], outs=[rs_outs[c].opt()],
                    )

        # ---- residual + LN2 + SwiGLU MLP (row-parallel) -------------
        with ExitStack() as mlp_scope:
            mpool = mlp_scope.enter_context(tc.tile_pool(name="mlp", bufs=1))
            wstream = mlp_scope.enter_context(tc.tile_pool(name="wstream", bufs=2))
            wdpool = mlp_scope.enter_context(tc.tile_pool(name="wdpool", bufs=2))
            psC = mlp_scope.enter_context(
                tc.tile_pool(name="psC", bufs=2, space="PSUM"))

            x2 = mpool.tile([128, RT, C], F32)
            y2T = mpool.tile([128, 8, ROWS], BF16)
            gu = mpool.tile([128, MCH, ROWS], BF16)

            for r in range(RT):
                rst = work.tile([128, C], BF16, tag="rst")
                nc.sync.dma_start(rst[:], rs_outs[r][:, :])
                xrt = work.tile([128, C], F32, tag="xrt")
                nc.sync.dma_start(xrt[:], xr_d[ts(r, 128), :])
                nc.vector.tensor_tensor(x2[:, r, :], rst[:], xrt[:],
                                        mybir.AluOpType.add)
                ssum = stats.tile([128, 1], F32, tag="ssum")
                nc.vector.reduce_sum(ssum[:], x2[:, r, :],
                                     axis=mybir.AxisListType.X)
                sq = work.tile([128, C], F32, tag="sq")
                ssq = stats.tile([128, 1], F32, tag="ssq")
                nc.scalar.activation(sq[:], x2[:, r, :],
                                     mybir.ActivationFunctionType.Square,
                                     accum_out=ssq[:])
                mu = stats.tile([128, 1], F32, tag="mu")
                nc.vector.tensor_scalar_mul(mu[:], ssum[:], 1.0 / C)
                ssqn = stats.tile([128, 1], F32, tag="ssqn")
                nc.vector.tensor_scalar_mul(ssqn[:], ssq[:], 1.0 / C)
                musq = stats.tile([128, 1], F32, tag="musq")
                nc.vector.tensor_tensor(musq[:], mu[:], mu[:],
                                        mybir.AluOpType.mult)
                var = stats.tile([128, 1], F32, tag="var")
                nc.vector.tensor_tensor(var[:], ssqn[:], musq[:],
                                        mybir.AluOpType.subtract)
                std = stats.tile([128, 1], F32, tag="std")
                nc.scalar.activation(std[:], var[:],
                                     mybir.ActivationFunctionType.Sqrt,
                                     bias=eps_ap[:])
                rstd = stats.tile([128, 1], F32, tag="rstd")
                nc.vector.reciprocal(rstd[:], std[:])
                yb = work.tile([128, C], BF16, tag="yb")
                nc.vector.tensor_scalar(yb[:], x2[:, r, :], mu[:], rstd[:],
                                        mybir.AluOpType.subtract,
                                        mybir.AluOpType.mult)
                for half in range(2):
                    pt = psC.tile([128, 4, 128], BF16, tag="tr2")
                    for i in range(4):
                        cc = half * 4 + i
                        nc.tensor.transpose(pt[:, i, :], yb[:, ts(cc, 128)],
                                            ident[:])
                    nc.vector.tensor_copy(
                        y2T[:, half * 4:(half + 1) * 4, ts(r, 128)], pt[:])

            for hc in range(MCH):
                wgt = wstream.tile([128, 8, 128], BF16, tag="wgt")
                nc.sync.dma_start(wgt[:], wg_d[:, :, ts(hc, 128)])
                wut = wstream.tile([128, 8, 128], BF16, tag="wut")
                nc.sync.dma_start(wut[:], wu_d[:, :, ts(hc, 128)])
                pg = psC.tile([128, 512], F32, tag="g")
                pu = psC.tile([128, 512], F32, tag="u")
                for cc in range(CCH):
                    nc.tensor.matmul(pg[:], wgt[:, cc, :], y2T[:, cc, :],
                                     start=(cc == 0), stop=(cc == CCH - 1))
                for cc in range(CCH):
                    nc.tensor.matmul(pu[:], wut[:, cc, :], y2T[:, cc, :],
                                     start=(cc == 0), stop=(cc == CCH - 1))
                gs = work.tile([128, 512], BF16, tag="gs")
                nc.scalar.activation(gs[:], pg[:],
                                     mybir.ActivationFunctionType.Silu,
                                     bias=bg_sb[:, hc:hc + 1])
                us = work.tile([128, 512], BF16, tag="us")
                nc.scalar.activation(us[:], pu[:],
                                     mybir.ActivationFunctionType.Identity,
                                     bias=bu_sb[:, hc:hc + 1])
                nc.vector.tensor_tensor(gu[:, hc, :], gs[:], us[:],
                                        mybir.AluOpType.mult)

            for nb in range(2):
                wdt = wdpool.tile([128, MCH, 512], BF16, tag="wdt")
                nc.sync.dma_start(wdt[:], wd_d[:, :, ts(nb, 512)])
                for tt in range(RT):
                    pd = psC.tile([128, 512], F32, tag="d")
                    for hc in range(MCH):
                        nc.tensor.matmul(pd[:], gu[:, hc, ts(tt, 128)],
                                         wdt[:, hc, :],
                                         start=(hc == 0), stop=(hc == MCH - 1))
                    o1 = work.tile([128, 512], F32, tag="o1")
                    nc.vector.tensor_tensor(o1[:], pd[:],
                                            x2[:, tt, ts(nb, 512)],
                                            mybir.AluOpType.add)
                    nc.vector.tensor_tensor(o1[:], o1[:],
                                            db_sb[:, ts(nb, 512)],
                                            mybir.AluOpType.add)
                    nc.sync.dma_start(out_d[ts(tt, 128), ts(nb, 512)], o1[:])

    _split_excess_waits(nc)
    return nc


# ----------------------------------------------------------------------------
# host-side input prep + launch
# ----------------------------------------------------------------------------
_cache = {}

def _get_nc(nqkv):
    if nqkv not in _cache:
        _cache[nqkv] = _build(nqkv)
    return _cache[nqkv]


def kernel(x, ln1_g, ln1_b, qkv_w, qkv_b, proj_w, proj_b,
           ln2_g, ln2_b, gate_w, gate_b, up_w, up_b, down_w, down_b):
    x = np.asarray(x, np.float32)
    f = lambda a: np.asarray(a, np.float32)
    ln1_g, ln1_b, qkv_b, proj_b, ln2_g, ln2_b = map(f, (
        ln1_g, ln1_b, qkv_b, proj_b, ln2_g, ln2_b))
    qkv_w, proj_w, gate_w, gate_b, up_w, up_b, down_w, down_b = map(f, (
        qkv_w, proj_w, gate_w, gate_b, up_w, up_b, down_w, down_b))

    slopes = _slopes()

    # fold LN affines into the consuming matmuls
    w1 = qkv_w * ln1_g[:, None]
    b1 = ln1_b @ qkv_w + qkv_b              # [3C]
    wg_f = gate_w * ln2_g[:, None]
    bg_f = ln2_b @ gate_w + gate_b          # [HID]
    wu_f = up_w * ln2_g[:, None]
    bu_f = ln2_b @ up_w + up_b

    nqkv = 9 if np.any(b1 != 0.0) else 8

    def qkv_aug(wcols, bcols, scale):
        # [C, 256] + bias row -> device layout [128, 2, 9, 128]
        wa = np.zeros((1152, 256), np.float32)
        wa[:C] = wcols * scale
        wa[C] = bcols * scale
        dev = np.zeros((128, 2, 9, 128), bf16)
        for p in range(2):
            blk = wa[:, p * 128:(p + 1) * 128]            # [1152, 128]
            dev[:, p] = blk.reshape(9, 128, 128).transpose(1, 0, 2).astype(bf16)
        return dev

    # masks (S^T diagonal tiles) and per-head exp biases
    pp = np.arange(128)[:, None]
    jj = np.arange(512)[None, :]
    masks_np = np.zeros((128, 4, 512), bf16)
    for v in range(4):
        masks_np[:, v, :] = (jj >= pp + 128 * v).astype(bf16)

    wgp = np.zeros((C, HIDP), np.float32); wgp[:, :HID] = wg_f
    wup = np.zeros((C, HIDP), np.float32); wup[:, :HID] = wu_f
    wdp = np.zeros((HIDP, 1024), np.float32); wdp[:HID] = down_w
    bgp = np.zeros(HIDP, np.float32); bgp[:HID] = bg_f
    bup = np.zeros(HIDP, np.float32); bup[:HID] = bu_f

    wg_dev = wgp.reshape(8, 128, HIDP).transpose(1, 0, 2).astype(bf16)
    wu_dev = wup.reshape(8, 128, HIDP).transpose(1, 0, 2).astype(bf16)
    wd_dev = wdp.reshape(MCH, 128, 1024).transpose(1, 0, 2).astype(bf16)
    bg_dev = bgp.reshape(MCH, 128).T.copy()
    bu_dev = bup.reshape(MCH, 128).T.copy()
    db_dev = np.broadcast_to(down_b, (128, 1024)).copy()

    in_maps = []
    for c in range(N_CORES):
        b, g = c // TP, c % TP
        heads = range(4 * g, 4 * g + 4)
        qcols = np.concatenate([np.arange(h * D, (h + 1) * D) for h in heads])
        kcols = qcols + C
        vcols = qcols + 2 * C

        wq_dev = qkv_aug(w1[:, qcols], b1[qcols], 0.125)
        wk_dev = qkv_aug(w1[:, kcols], b1[kcols], 1.0)
        wv_a = np.zeros((1152, 256), np.float32)
        wv_a[:C] = w1[:, vcols]
        wv_a[C] = b1[vcols]
        wv_dev = wv_a.reshape(9, 128, 256).transpose(1, 0, 2).astype(bf16)

        wp_rows = proj_w[qcols, :]                        # [256, 1024]
        wp_dev = wp_rows.reshape(2, 128, 1024).transpose(1, 0, 2).astype(bf16)

        eb = np.zeros((128, HL * 16), np.float32)
        for hl, h in enumerate(heads):
            sl = slopes[h]
            for t in range(16):
                eb[:, hl * 16 + t] = (-sl * (128 * t + np.arange(128))
                                      - CBIAS).astype(np.float32)

        in_maps.append({
            "x_full": x[b],
            "x_rows": np.concatenate(
                [x[b, 512 * r + 128 * g:512 * r + 128 * g + 128]
                 for r in range(4)], axis=0) + proj_b[None, :],
            "wq": wq_dev, "wk": wk_dev, "wv": wv_dev, "wproj": wp_dev,
            "wg": wg_dev, "wu": wu_dev, "wd": wd_dev,
            "bg": bg_dev, "bu": bu_dev, "dbias": db_dev,
            "masks": masks_np, "ebias": eb,
        })

    nc = _get_nc(nqkv)
    res = bass_utils.run_bass_kernel_spmd(
        nc, in_maps, core_ids=list(range(N_CORES)))

    out = np.empty((B, T, C), np.float32)
    for c in range(N_CORES):
        b, g = c // TP, c % TP
        orr = res.results[c]["out_rows"]
        for r in range(4):
            out[b, 512 * r + 128 * g:512 * r + 128 * g + 128] = \
                orr[128 * r:128 * (r + 1)]
    return out


# revision 7
# speedup vs baseline: 1.2521x; 1.0070x over previous
"""Trainium2 Bass kernel for EnhancedTransformerBlock (B=2,T=2048,C=1024,H=16,
SwiGLU HIDDEN=2730, ALiBi-abs + causal attention).

Sharding over 8 cores: batch (2) x head-groups (4 heads/core). Attention is
computed head-parallel with transposed scores S^T[tk,tq] so ALiBi + max-shift
collapse into a per-partition exp bias; softmax denominator rides as a ones
column in V through the PV matmul. A ReduceScatter over each batch group
converts the proj partials to row-sharded activations; the SwiGLU MLP then
runs row-parallel with full weights. Host reassembles row blocks.
"""
import sys, types
sys.path.insert(0, "/opt/trn_rl_repo")
import numpy as np
import ml_dtypes

import concourse.bass as bass
import concourse.tile as tile
from concourse import mybir
import concourse.bass_utils as bass_utils
import bass_rust
from concourse.masks import make_identity

# ----------------------------------------------------------------------------
# environment patches (walrus in this container accepts only 1 sync-wait/inst)
# ----------------------------------------------------------------------------
_DRAIN_WAIT_LIMIT = 1

def _patched_drain_and_barrier(self, tick_clock, wait_clock):
    nc = self.nc
    drain_inst = nc.sync.drain()
    wait_clock.add_sem_waits(
        drain_inst.ins, bass_rust.ScopedClock({None: tick_clock.global_clock})
    )
    si = drain_inst.ins.sync_info
    waits = list(si.on_wait) if si is not None else []
    if len(waits) > _DRAIN_WAIT_LIMIT:
        si.on_wait = waits[:_DRAIN_WAIT_LIMIT]
        for i in range(_DRAIN_WAIT_LIMIT, len(waits), _DRAIN_WAIT_LIMIT):
            d2 = nc.sync.drain()
            d2.ins.sync_info = bass_rust.SyncInfo(
                on_wait=waits[i:i + _DRAIN_WAIT_LIMIT], on_update=[]
            )
    nc.all_engine_barrier()
    popped = nc._tile_sem_poison_stack.pop()
    assert popped is self._sem_poison
    nc.clear_and_free_semaphores(list(self.sems.allocated().values()))
    nc.all_engine_barrier()


def _split_excess_waits(nc, limit=_DRAIN_WAIT_LIMIT):
    n = [0]
    for bb in nc.main_func.blocks:
        insts = bb.instructions
        out = []
        changed = False
        for inst in insts:
            si = inst.sync_info
            waits = list(si.on_wait) if si is not None else []
            if len(waits) > limit:
                changed = True
                keep = waits[-limit:]
                rest = waits[:-limit]
                for i in range(0, len(rest), limit):
                    n[0] += 1
                    d = mybir.InstNoOp(
                        name=f"waitsplit-{n[0]}", engine=inst.engine, ins=[], outs=[]
                    )
                    d.sync_info = bass_rust.SyncInfo(
                        on_wait=rest[i:i + limit], on_update=[]
                    )
                    out.append(d)
                si.on_wait = keep
            out.append(inst)
        if changed:
            bb.instructions = out


def _install_patches():
    tile.TileContext._drain_and_barrier = _patched_drain_and_barrier
    if "antenv.axon_hooks" not in sys.modules:
        try:
            from trn_agent_boot.trn_boot import _ntff_profile_via_ctypes
            hook = _ntff_profile_via_ctypes("/opt/axon/libaxon_pjrt.so")
        except Exception:
            hook = None
        mod = types.ModuleType("antenv.axon_hooks")
        mod.get_axon_ntff_profile_hook = lambda: hook
        mod.set_axon_ntff_profile_hook = lambda h: None
        sys.modules["antenv.axon_hooks"] = mod
        bass_utils.upload_artifacts = lambda tmpdir: tmpdir

_install_patches()

# ----------------------------------------------------------------------------
# problem constants
# ----------------------------------------------------------------------------
B, T, C = 2, 2048, 1024
H, D = 16, 64
HID, HIDP = 2730, 2816
N_CORES, TP = 8, 4
ROWS = T // TP          # 512 rows per core after reduce-scatter
HL = 4                  # local heads per core
EPS = 1e-5
CBIAS = 24.0
NEG = -1e30
F32, BF16 = mybir.dt.float32, mybir.dt.bfloat16
bf16 = ml_dtypes.bfloat16
ts = bass.ts

NTT = T // 128          # 16 token tiles
NTB = T // 512          # 4 token blocks
CCH = C // 128          # 8 feature chunks
MCH = HIDP // 128       # 22 hidden chunks
RT = ROWS // 128        # 4 row tiles per core


def _slopes():
    i = np.arange(1, H + 1, dtype=np.float64)
    return (1.0 / np.power(2.0, 8.0 * i / H)).astype(np.float64)


# ----------------------------------------------------------------------------
# device program (identical on all 8 cores; per-core behavior via input data)
# ----------------------------------------------------------------------------
def _build(nqkv):
    nc = bass.Bass("TRN2", num_devices=N_CORES)

    x_d = nc.dram_tensor("x_full", [T, C], F32, kind="ExternalInput")
    xr_d = nc.dram_tensor("x_rows", [ROWS, C], F32, kind="ExternalInput")
    wq_d = nc.dram_tensor("wq", [128, 2, 9, 128], BF16, kind="ExternalInput")
    wk_d = nc.dram_tensor("wk", [128, 2, 9, 128], BF16, kind="ExternalInput")
    wv_d = nc.dram_tensor("wv", [128, 9, 256], BF16, kind="ExternalInput")
    wp_d = nc.dram_tensor("wproj", [128, 2, 1024], BF16, kind="ExternalInput")
    wg_d = nc.dram_tensor("wg", [128, 8, HIDP], BF16, kind="ExternalInput")
    wu_d = nc.dram_tensor("wu", [128, 8, HIDP], BF16, kind="ExternalInput")
    wd_d = nc.dram_tensor("wd", [128, MCH, 1024], BF16, kind="ExternalInput")
    bg_d = nc.dram_tensor("bg", [128, MCH], F32, kind="ExternalInput")
    bu_d = nc.dram_tensor("bu", [128, MCH], F32, kind="ExternalInput")
    db_d = nc.dram_tensor("dbias", [128, 1024], F32, kind="ExternalInput")
    mk_d = nc.dram_tensor("masks", [128, 4, 512], BF16, kind="ExternalInput")
    eb_d = nc.dram_tensor("ebias", [128, HL * 16], F32, kind="ExternalInput")

    out_d = nc.dram_tensor("out_rows", [ROWS, C], F32, kind="ExternalOutput")
    scr_d = nc.dram_tensor("scr", [16, 512], F32, kind="ExternalOutput")

    from contextlib import ExitStack
    with tile.TileContext(nc) as tc, ExitStack() as top:
        consts = top.enter_context(tc.tile_pool(name="consts", bufs=1))
        stats = top.enter_context(tc.tile_pool(name="stats", bufs=16))
        work = top.enter_context(tc.tile_pool(name="work", bufs=2))
        dramp = top.enter_context(tc.tile_pool(name="dram", bufs=1, space="DRAM"))

        ident = consts.tile([128, 128], BF16)
        make_identity(nc, ident[:])
        eps_ap = consts.tile([128, 1], F32)
        nc.vector.memset(eps_ap[:], EPS)
        masks = consts.tile([128, 4, 512], BF16)
        nc.gpsimd.dma_start(masks[:], mk_d[:, :, :])
        ebias = consts.tile([128, HL * 16], F32)
        nc.gpsimd.dma_start(ebias[:], eb_d[:, :])
        bg_sb = consts.tile([128, MCH], F32)
        nc.gpsimd.dma_start(bg_sb[:], bg_d[:, :])
        bu_sb = consts.tile([128, MCH], F32)
        nc.gpsimd.dma_start(bu_sb[:], bu_d[:, :])
        db_sb = consts.tile([128, 1024], F32)
        nc.gpsimd.dma_start(db_sb[:], db_d[:, :])

        rs_ins = [dramp.tile([512, C], BF16, name=f"rsin{i}") for i in range(NTB)]
        rs_outs = [dramp.tile([128, C], BF16, name=f"rsout{i}") for i in range(NTB)]

        with ExitStack() as attn_scope:
            apool = attn_scope.enter_context(tc.tile_pool(name="attn", bufs=1))
            pipe = attn_scope.enter_context(tc.tile_pool(name="pipe", bufs=4))
            OnTp = attn_scope.enter_context(tc.tile_pool(name="OnTp", bufs=2))

            wq_sb = apool.tile([128, 2, 9, 128], BF16)
            nc.gpsimd.dma_start(wq_sb[:], wq_d[:, :, :, :])
            wk_sb = apool.tile([128, 2, 9, 128], BF16)
            nc.gpsimd.dma_start(wk_sb[:], wk_d[:, :, :, :])
            wv_sb = apool.tile([128, 9, 256], BF16)
            nc.gpsimd.dma_start(wv_sb[:], wv_d[:, :, :])
            wp_sb = apool.tile([128, 2, 1024], BF16)
            nc.gpsimd.dma_start(wp_sb[:], wp_d[:, :, :])

            yT = apool.tile([128, nqkv, T], BF16)
            qT = apool.tile([128, HL, T], BF16)
            kT = apool.tile([128, HL, T], BF16)
            Vh = apool.tile([128, NTT, HL, 66], BF16)

            nc.vector.memset(qT[:], 0.0)
            nc.vector.memset(kT[:], 0.0)
            nc.scalar.memzero(Vh[:, :, :, 64:66])
            nc.vector.memset(Vh[:, :, :, 64:65], 1.0)
            if nqkv == 9:
                nc.vector.memset(yT[:, 8, :], 0.0)
                nc.vector.memset(yT[0:1, 8, :], 1.0)

            with ExitStack() as qkv_scope:
                psA = qkv_scope.enter_context(
                    tc.tile_pool(name="psA", bufs=2, space="PSUM"))

                # ---- LN1 + transpose to feature-major yT ----------------
                for t in range(NTT):
                    xt = work.tile([128, C], F32, tag="xt")
                    nc.sync.dma_start(xt[:], x_d[ts(t, 128), :])
                    bst = stats.tile([128, 2, 6], F32, tag="bst")
                    for sg in range(2):
                        nc.vector.bn_stats(bst[:, sg, :], xt[:, ts(sg, 512)])
                    mv = stats.tile([128, 2], F32, tag="mv")
                    nc.vector.bn_aggr(mv[:], bst[:])
                    std = stats.tile([128, 1], F32, tag="std")
                    nc.scalar.activation(std[:], mv[:, 1:2],
                                         mybir.ActivationFunctionType.Sqrt,
                                         bias=eps_ap[:])
                    rstd = stats.tile([128, 1], F32, tag="rstd")
                    nc.vector.reciprocal(rstd[:], std[:])
                    yb = work.tile([128, C], BF16, tag="yb")
                    nc.vector.tensor_scalar(yb[:], xt[:], mv[:, 0:1], rstd[:],
                                            mybir.AluOpType.subtract,
                                            mybir.AluOpType.mult)
                    for half in range(2):
                        pt = psA.tile([128, 4, 128], BF16, tag="tr")
                        for i in range(4):
                            cc = half * 4 + i
                            nc.tensor.transpose(pt[:, i, :],
                                                yb[:, ts(cc, 128)], ident[:])
                        nc.vector.tensor_copy(
                            yT[:, half * 4:(half + 1) * 4, ts(t, 128)], pt[:])

                # ---- QKV ------------------------------------------------
                for p in range(2):      # head pairs -> q/k feature-major
                    for blk in range(NTB):
                        for wsb, dstT in ((wq_sb, qT), (wk_sb, kT)):
                            ps = psA.tile([128, 512], F32, tag="qkv")
                            for cc in range(nqkv):
                                nc.tensor.matmul(ps[:], wsb[:, p, cc, :],
                                                 yT[:, cc, ts(blk, 512)],
                                                 start=(cc == 0),
                                                 stop=(cc == nqkv - 1))
                            nc.vector.tensor_copy(
                                dstT[0:64, 2 * p, ts(blk, 512)], ps[0:64, :])
                            nc.vector.tensor_copy(
                                dstT[0:64, 2 * p + 1, ts(blk, 512)],
                                ps[64:128, :])
                for t in range(NTT):    # V token-major
                    ps = psA.tile([128, 256], F32, tag="qkv")
                    for cc in range(nqkv):
                        nc.tensor.matmul(ps[:], yT[:, cc, ts(t, 128)],
                                         wv_sb[:, cc, :],
                                         start=(cc == 0), stop=(cc == nqkv - 1))
                    nc.scalar.copy(Vh[:, t, :, 0:64],
                                   ps.rearrange("p (h d) -> p h d", d=64))

            # ---- attention + per-block proj + pipelined RS --------------
            with ExitStack() as ps_scope:
                psS = ps_scope.enter_context(
                    tc.tile_pool(name="psS", bufs=2, space="PSUM"))
                psO = ps_scope.enter_context(
                    tc.tile_pool(name="psO", bufs=3, space="PSUM"))
                psJ = ps_scope.enter_context(
                    tc.tile_pool(name="psJ", bufs=2, space="PSUM"))
                for c in range(NTB):
                    OnT_c = OnTp.tile([128, 2, 512], BF16, tag="OnT")
                    for h in range(HL):
                        po = psO.tile([128, 512], F32, tag="o")
                        t_hi = 4 * c + 4
                        for t in range(t_hi):
                            s = psS.tile([128, 512], F32, tag="s")
                            nc.tensor.matmul(s[:], kT[:, h, ts(t, 128)],
                                             qT[:, h, ts(c, 512)],
                                             start=True, stop=True)
                            pT = pipe.tile([128, 512], BF16, tag="pT")
                            idx = h * 16 + t
                            nc.scalar.activation(
                                pT[:], s[:], mybir.ActivationFunctionType.Exp,
                                bias=ebias[:, idx:idx + 1])
                            if t >= 4 * c:
                                nc.vector.tensor_tensor(
                                    pT[:], pT[:], masks[:, t - 4 * c, :],
                                    mybir.AluOpType.mult)
                            nc.tensor.matmul(po[0:66, :], Vh[:, t, h, 0:66],
                                             pT[:], start=(t == 0),
                                             stop=(t == t_hi - 1))
                        i = h * 4 + c
                        den = pipe.tile([1, 512], F32, tag="den")
                        nc.vector.tensor_copy(den[:], po[64:65, :])
                        otmp = pipe.tile([64, 512], F32, tag="otmp")
                        nc.vector.tensor_copy(otmp[:], po[0:64, :])
                        nc.sync.dma_start(scr_d[i:i + 1, :], den[0:1, :])
                        denb = pipe.tile([64, 512], F32, tag="denb")
                        nc.sync.dma_start(
                            denb[:], scr_d[i:i + 1, :].to_broadcast((64, 512)))
                        rec = pipe.tile([64, 512], F32, tag="rec")
                        nc.vector.reciprocal(rec[:], denb[:])
                        nc.vector.tensor_tensor(
                            OnT_c[(h % 2) * 64:(h % 2) * 64 + 64, h // 2, :],
                            otmp[:], rec[:], mybir.AluOpType.mult)
                    # proj partial for this token block -> reduce-scatter
                    for i2 in range(4):
                        for nb in range(2):
                            pp_ = psJ.tile([128, 512], F32, tag="pj")
                            for cc in range(2):
                                nc.tensor.matmul(pp_[:],
                                                 OnT_c[:, cc, ts(i2, 128)],
                                                 wp_sb[:, cc, ts(nb, 512)],
                                                 start=(cc == 0), stop=(cc == 1))
                            pjs = pipe.tile([128, 512], BF16, tag="pjs")
                            nc.scalar.copy(pjs[:], pp_[:])
                            nc.sync.dma_start(
                                rs_ins[c][ts(i2, 128), ts(nb, 512)], pjs[:])
                    nc.gpsimd.collective_compute(
                        "ReduceScatter", mybir.AluOpType.add,
                        replica_groups=[[0, 1, 2, 3], [4, 5, 6, 7]],
                        ins=[rs_ins[c].opt()], outs=[rs_outs[c].opt()],
                    )

        # ---- residual + LN2 + SwiGLU MLP (row-parallel) -------------
        with ExitStack() as mlp_scope:
            mpool = mlp_scope.enter_context(tc.tile_pool(name="mlp", bufs=1))
            wstream = mlp_scope.enter_context(tc.tile_pool(name="wstream", bufs=2))
            wdpool = mlp_scope.enter_context(tc.tile_pool(name="wdpool", bufs=2))
            psC = mlp_scope.enter_context(
                tc.tile_pool(name="psC", bufs=2, space="PSUM"))

            x2 = mpool.tile([128, RT, C], F32)
            y2T = mpool.tile([128, 8, ROWS], BF16)
            gu = mpool.tile([128, MCH, ROWS], BF16)

            for r in range(RT):
                rst = work.tile([128, C], BF16, tag="rst")
                nc.sync.dma_start(rst[:], rs_outs[r][:, :])
                xrt = work.tile([128, C], F32, tag="xrt")
                nc.sync.dma_start(xrt[:], xr_d[ts(r, 128), :])
                nc.vector.tensor_tensor(x2[:, r, :], rst[:], xrt[:],
                                        mybir.AluOpType.add)
                bst = stats.tile([128, 2, 6], F32, tag="bst")
                for sg in range(2):
                    nc.vector.bn_stats(bst[:, sg, :], x2[:, r, ts(sg, 512)])
                mv = stats.tile([128, 2], F32, tag="mv")
                nc.vector.bn_aggr(mv[:], bst[:])
                std = stats.tile([128, 1], F32, tag="std")
                nc.scalar.activation(std[:], mv[:, 1:2],
                                     mybir.ActivationFunctionType.Sqrt,
                                     bias=eps_ap[:])
                rstd = stats.tile([128, 1], F32, tag="rstd")
                nc.vector.reciprocal(rstd[:], std[:])
                yb = work.tile([128, C], BF16, tag="yb")
                nc.vector.tensor_scalar(yb[:], x2[:, r, :], mv[:, 0:1], rstd[:],
                                        mybir.AluOpType.subtract,
                                        mybir.AluOpType.mult)
                for half in range(2):
                    pt = psC.tile([128, 4, 128], BF16, tag="tr2")
                    for i in range(4):
                        cc = half * 4 + i
                        nc.tensor.transpose(pt[:, i, :], yb[:, ts(cc, 128)],
                                            ident[:])
                    nc.vector.tensor_copy(
                        y2T[:, half * 4:(half + 1) * 4, ts(r, 128)], pt[:])

            for hc in range(MCH):
                wgt = wstream.tile([128, 8, 128], BF16, tag="wgt")
                nc.gpsimd.dma_start(wgt[:], wg_d[:, :, ts(hc, 128)])
                wut = wstream.tile([128, 8, 128], BF16, tag="wut")
                nc.gpsimd.dma_start(wut[:], wu_d[:, :, ts(hc, 128)])
                pg = psC.tile([128, 512], F32, tag="g")
                pu = psC.tile([128, 512], F32, tag="u")
                for cc in range(CCH):
                    nc.tensor.matmul(pg[:], wgt[:, cc, :], y2T[:, cc, :],
                                     start=(cc == 0), stop=(cc == CCH - 1))
                for cc in range(CCH):
                    nc.tensor.matmul(pu[:], wut[:, cc, :], y2T[:, cc, :],
                                     start=(cc == 0), stop=(cc == CCH - 1))
                gs = work.tile([128, 512], BF16, tag="gs")
                nc.scalar.activation(gs[:], pg[:],
                                     mybir.ActivationFunctionType.Silu,
                                     bias=bg_sb[:, hc:hc + 1])
                us = work.tile([128, 512], BF16, tag="us")
                nc.scalar.activation(us[:], pu[:],
                                     mybir.ActivationFunctionType.Identity,
                                     bias=bu_sb[:, hc:hc + 1])
                nc.vector.tensor_tensor(gu[:, hc, :], gs[:], us[:],
                                        mybir.AluOpType.mult)

            for nb in range(2):
                wdt = wdpool.tile([128, MCH, 512], BF16, tag="wdt")
                nc.gpsimd.dma_start(wdt[:], wd_d[:, :, ts(nb, 512)])
                for tt in range(RT):
                    pd = psC.tile([128, 512], F32, tag="d")
                    for hc in range(MCH):
                        nc.tensor.matmul(pd[:], gu[:, hc, ts(tt, 128)],
                                         wdt[:, hc, :],
                                         start=(hc == 0), stop=(hc == MCH - 1))
                    o1 = work.tile([128, 512], F32, tag="o1")
                    nc.vector.tensor_tensor(o1[:], pd[:],
                                            x2[:, tt, ts(nb, 512)],
                                            mybir.AluOpType.add)
                    nc.vector.tensor_tensor(o1[:], o1[:],
                                            db_sb[:, ts(nb, 512)],
                                            mybir.AluOpType.add)
                    nc.sync.dma_start(out_d[ts(tt, 128), ts(nb, 512)], o1[:])

    _split_excess_waits(nc)
    return nc


# ----------------------------------------------------------------------------
# host-side input prep + launch
# ----------------------------------------------------------------------------
_cache = {}

def _get_nc(nqkv):
    if nqkv not in _cache:
        _cache[nqkv] = _build(nqkv)
    return _cache[nqkv]


def kernel(x, ln1_g, ln1_b, qkv_w, qkv_b, proj_w, proj_b,
           ln2_g, ln2_b, gate_w, gate_b, up_w, up_b, down_w, down_b):
    x = np.asarray(x, np.float32)
    f = lambda a: np.asarray(a, np.float32)
    ln1_g, ln1_b, qkv_b, proj_b, ln2_g, ln2_b = map(f, (
        ln1_g, ln1_b, qkv_b, proj_b, ln2_g, ln2_b))
    qkv_w, proj_w, gate_w, gate_b, up_w, up_b, down_w, down_b = map(f, (
        qkv_w, proj_w, gate_w, gate_b, up_w, up_b, down_w, down_b))

    slopes = _slopes()

    # fold LN affines into the consuming matmuls
    w1 = qkv_w * ln1_g[:, None]
    b1 = ln1_b @ qkv_w + qkv_b              # [3C]
    wg_f = gate_w * ln2_g[:, None]
    bg_f = ln2_b @ gate_w + gate_b          # [HID]
    wu_f = up_w * ln2_g[:, None]
    bu_f = ln2_b @ up_w + up_b

    nqkv = 9 if np.any(b1 != 0.0) else 8

    def qkv_aug(wcols, bcols, scale):
        # [C, 256] + bias row -> device layout [128, 2, 9, 128]
        wa = np.zeros((1152, 256), np.float32)
        wa[:C] = wcols * scale
        wa[C] = bcols * scale
        dev = np.zeros((128, 2, 9, 128), bf16)
        for p in range(2):
            blk = wa[:, p * 128:(p + 1) * 128]            # [1152, 128]
            dev[:, p] = blk.reshape(9, 128, 128).transpose(1, 0, 2).astype(bf16)
        return dev

    # masks (S^T diagonal tiles) and per-head exp biases
    pp = np.arange(128)[:, None]
    jj = np.arange(512)[None, :]
    masks_np = np.zeros((128, 4, 512), bf16)
    for v in range(4):
        masks_np[:, v, :] = (jj >= pp + 128 * v).astype(bf16)

    wgp = np.zeros((C, HIDP), np.float32); wgp[:, :HID] = wg_f
    wup = np.zeros((C, HIDP), np.float32); wup[:, :HID] = wu_f
    wdp = np.zeros((HIDP, 1024), np.float32); wdp[:HID] = down_w
    bgp = np.zeros(HIDP, np.float32); bgp[:HID] = bg_f
    bup = np.zeros(HIDP, np.float32); bup[:HID] = bu_f

    wg_dev = wgp.reshape(8, 128, HIDP).transpose(1, 0, 2).astype(bf16)
    wu_dev = wup.reshape(8, 128, HIDP).transpose(1, 0, 2).astype(bf16)
    wd_dev = wdp.reshape(MCH, 128, 1024).transpose(1, 0, 2).astype(bf16)
    bg_dev = bgp.reshape(MCH, 128).T.copy()
    bu_dev = bup.reshape(MCH, 128).T.copy()
    db_dev = np.broadcast_to(down_b, (128, 1024)).copy()

    in_maps = []
    for c in range(N_CORES):
        b, g = c // TP, c % TP
        heads = range(4 * g, 4 * g + 4)
        qcols = np.concatenate([np.arange(h * D, (h + 1) * D) for h in heads])
        kcols = qcols + C
        vcols = qcols + 2 * C

        wq_dev = qkv_aug(w1[:, qcols], b1[qcols], 0.125)
        wk_dev = qkv_aug(w1[:, kcols], b1[kcols], 1.0)
        wv_a = np.zeros((1152, 256), np.float32)
        wv_a[:C] = w1[:, vcols]
        wv_a[C] = b1[vcols]
        wv_dev = wv_a.reshape(9, 128, 256).transpose(1, 0, 2).astype(bf16)

        wp_rows = proj_w[qcols, :]                        # [256, 1024]
        wp_dev = wp_rows.reshape(2, 128, 1024).transpose(1, 0, 2).astype(bf16)

        eb = np.zeros((128, HL * 16), np.float32)
        for hl, h in enumerate(heads):
            sl = slopes[h]
            for t in range(16):
                eb[:, hl * 16 + t] = (-sl * (128 * t + np.arange(128))
                                      - CBIAS).astype(np.float32)

        in_maps.append({
            "x_full": x[b],
            "x_rows": np.concatenate(
                [x[b, 512 * r + 128 * g:512 * r + 128 * g + 128]
                 for r in range(4)], axis=0) + proj_b[None, :],
            "wq": wq_dev, "wk": wk_dev, "wv": wv_dev, "wproj": wp_dev,
            "wg": wg_dev, "wu": wu_dev, "wd": wd_dev,
            "bg": bg_dev, "bu": bu_dev, "dbias": db_dev,
            "masks": masks_np, "ebias": eb,
        })

    nc = _get_nc(nqkv)
    res = bass_utils.run_bass_kernel_spmd(
        nc, in_maps, core_ids=list(range(N_CORES)))

    out = np.empty((B, T, C), np.float32)
    for c in range(N_CORES):
        b, g = c // TP, c % TP
        orr = res.results[c]["out_rows"]
        for r in range(4):
            out[b, 512 * r + 128 * g:512 * r + 128 * g + 128] = \
                orr[128 * r:128 * (r + 1)]
    return out


# revision 11
# speedup vs baseline: 1.2646x; 1.0099x over previous
"""Trainium2 Bass kernel for EnhancedTransformerBlock (B=2,T=2048,C=1024,H=16,
SwiGLU HIDDEN=2730, ALiBi-abs + causal attention).

Sharding over 8 cores: batch (2) x head-groups (4 heads/core). Attention is
computed head-parallel with transposed scores S^T[tk,tq] so ALiBi + max-shift
collapse into a per-partition exp bias; softmax denominator rides as a ones
column in V through the PV matmul. A ReduceScatter over each batch group
converts the proj partials to row-sharded activations; the SwiGLU MLP then
runs row-parallel with full weights. Host reassembles row blocks.
"""
import sys, types
sys.path.insert(0, "/opt/trn_rl_repo")
import numpy as np
import ml_dtypes

import concourse.bass as bass
import concourse.tile as tile
from concourse import mybir
import concourse.bass_utils as bass_utils
import bass_rust
from concourse.masks import make_identity

# ----------------------------------------------------------------------------
# environment patches (walrus in this container accepts only 1 sync-wait/inst)
# ----------------------------------------------------------------------------
_DRAIN_WAIT_LIMIT = 1

def _patched_drain_and_barrier(self, tick_clock, wait_clock):
    nc = self.nc
    drain_inst = nc.sync.drain()
    wait_clock.add_sem_waits(
        drain_inst.ins, bass_rust.ScopedClock({None: tick_clock.global_clock})
    )
    si = drain_inst.ins.sync_info
    waits = list(si.on_wait) if si is not None else []
    if len(waits) > _DRAIN_WAIT_LIMIT:
        si.on_wait = waits[:_DRAIN_WAIT_LIMIT]
        for i in range(_DRAIN_WAIT_LIMIT, len(waits), _DRAIN_WAIT_LIMIT):
            d2 = nc.sync.drain()
            d2.ins.sync_info = bass_rust.SyncInfo(
                on_wait=waits[i:i + _DRAIN_WAIT_LIMIT], on_update=[]
            )
    nc.all_engine_barrier()
    popped = nc._tile_sem_poison_stack.pop()
    assert popped is self._sem_poison
    nc.clear_and_free_semaphores(list(self.sems.allocated().values()))
    nc.all_engine_barrier()


def _split_excess_waits(nc, limit=_DRAIN_WAIT_LIMIT):
    n = [0]
    for bb in nc.main_func.blocks:
        insts = bb.instructions
        out = []
        changed = False
        for inst in insts:
            si = inst.sync_info
            waits = list(si.on_wait) if si is not None else []
            if len(waits) > limit:
                changed = True
                keep = waits[-limit:]
                rest = waits[:-limit]
                for i in range(0, len(rest), limit):
                    n[0] += 1
                    d = mybir.InstNoOp(
                        name=f"waitsplit-{n[0]}", engine=inst.engine, ins=[], outs=[]
                    )
                    d.sync_info = bass_rust.SyncInfo(
                        on_wait=rest[i:i + limit], on_update=[]
                    )
                    out.append(d)
                si.on_wait = keep
            out.append(inst)
        if changed:
            bb.instructions = out


def _install_patches():
    tile.TileContext._drain_and_barrier = _patched_drain_and_barrier
    if "antenv.axon_hooks" not in sys.modules:
        try:
            from trn_agent_boot.trn_boot import _ntff_profile_via_ctypes
            hook = _ntff_profile_via_ctypes("/opt/axon/libaxon_pjrt.so")
        except Exception:
            hook = None
        mod = types.ModuleType("antenv.axon_hooks")
        mod.get_axon_ntff_profile_hook = lambda: hook
        mod.set_axon_ntff_profile_hook = lambda h: None
        sys.modules["antenv.axon_hooks"] = mod
        bass_utils.upload_artifacts = lambda tmpdir: tmpdir

_install_patches()

# ----------------------------------------------------------------------------
# problem constants
# ----------------------------------------------------------------------------
B, T, C = 2, 2048, 1024
H, D = 16, 64
HID, HIDP = 2730, 2816
N_CORES, TP = 8, 4
ROWS = T // TP          # 512 rows per core after reduce-scatter
HL = 4                  # local heads per core
EPS = 1e-5
CBIAS = 24.0
NEG = -1e30
F32, BF16 = mybir.dt.float32, mybir.dt.bfloat16
F32R = mybir.dt.float32r
bf16 = ml_dtypes.bfloat16
ts = bass.ts

NTT = T // 128          # 16 token tiles
NTB = T // 512          # 4 token blocks
CCH = C // 128          # 8 feature chunks
MCH = HIDP // 128       # 22 hidden chunks
RT = ROWS // 128        # 4 row tiles per core


def _slopes():
    i = np.arange(1, H + 1, dtype=np.float64)
    return (1.0 / np.power(2.0, 8.0 * i / H)).astype(np.float64)


# ----------------------------------------------------------------------------
# device program (identical on all 8 cores; per-core behavior via input data)
# ----------------------------------------------------------------------------
def _build(nqkv):
    nc = bass.Bass("TRN2", num_devices=N_CORES)

    x_d = nc.dram_tensor("x_full", [T, C], F32, kind="ExternalInput")
    xr_d = nc.dram_tensor("x_rows", [ROWS, C], F32, kind="ExternalInput")
    wq_d = nc.dram_tensor("wq", [128, 2, 9, 128], BF16, kind="ExternalInput")
    wk_d = nc.dram_tensor("wk", [128, 2, 9, 128], BF16, kind="ExternalInput")
    wv_d = nc.dram_tensor("wv", [128, 9, 256], BF16, kind="ExternalInput")
    wp_d = nc.dram_tensor("wproj", [128, 2, 1024], BF16, kind="ExternalInput")
    wg_d = nc.dram_tensor("wg", [128, 8, HIDP], BF16, kind="ExternalInput")
    wu_d = nc.dram_tensor("wu", [128, 8, HIDP], BF16, kind="ExternalInput")
    wd_d = nc.dram_tensor("wd", [128, MCH, 1024], BF16, kind="ExternalInput")
    bg_d = nc.dram_tensor("bg", [128, MCH], F32, kind="ExternalInput")
    bu_d = nc.dram_tensor("bu", [128, MCH], F32, kind="ExternalInput")
    db_d = nc.dram_tensor("dbias", [128, 1024], F32, kind="ExternalInput")
    mk_d = nc.dram_tensor("masks", [128, 4, 512], BF16, kind="ExternalInput")
    eb_d = nc.dram_tensor("ebias", [128, HL * 16], F32, kind="ExternalInput")

    out_d = nc.dram_tensor("out_rows", [ROWS, C], F32, kind="ExternalOutput")
    scr_d = nc.dram_tensor("scr", [16, 512], F32, kind="ExternalOutput")

    from contextlib import ExitStack
    with tile.TileContext(nc) as tc, ExitStack() as top:
        consts = top.enter_context(tc.tile_pool(name="consts", bufs=1))
        stats = top.enter_context(tc.tile_pool(name="stats", bufs=16))
        work = top.enter_context(tc.tile_pool(name="work", bufs=2))
        dramp = top.enter_context(tc.tile_pool(name="dram", bufs=1, space="DRAM"))

        ident = consts.tile([128, 128], BF16)
        make_identity(nc, ident[:])
        eps_ap = consts.tile([128, 1], F32)
        nc.vector.memset(eps_ap[:], EPS)
        masks = consts.tile([128, 4, 512], BF16)
        nc.gpsimd.dma_start(masks[:], mk_d[:, :, :])
        ebias = consts.tile([128, HL * 16], F32)
        nc.gpsimd.dma_start(ebias[:], eb_d[:, :])
        bg_sb = consts.tile([128, MCH], F32)
        nc.gpsimd.dma_start(bg_sb[:], bg_d[:, :])
        bu_sb = consts.tile([128, MCH], F32)
        nc.gpsimd.dma_start(bu_sb[:], bu_d[:, :])
        db_sb = consts.tile([128, 1024], F32)
        nc.gpsimd.dma_start(db_sb[:], db_d[:, :])

        rs_ins = [dramp.tile([512, C], BF16, name=f"rsin{i}") for i in range(NTB)]
        rs_outs = [dramp.tile([128, C], BF16, name=f"rsout{i}") for i in range(NTB)]

        with ExitStack() as attn_scope:
            apool = attn_scope.enter_context(tc.tile_pool(name="attn", bufs=1))
            pipe = attn_scope.enter_context(tc.tile_pool(name="pipe", bufs=4))
            OnTp = attn_scope.enter_context(tc.tile_pool(name="OnTp", bufs=2))

            wq_sb = apool.tile([128, 2, 9, 128], BF16)
            nc.gpsimd.dma_start(wq_sb[:], wq_d[:, :, :, :])
            wk_sb = apool.tile([128, 2, 9, 128], BF16)
            nc.gpsimd.dma_start(wk_sb[:], wk_d[:, :, :, :])
            wv_sb = apool.tile([128, 9, 256], BF16)
            nc.gpsimd.dma_start(wv_sb[:], wv_d[:, :, :])
            wp_sb = apool.tile([128, 2, 1024], BF16)
            nc.gpsimd.dma_start(wp_sb[:], wp_d[:, :, :])

            yT = apool.tile([128, nqkv, T], BF16)
            qT = apool.tile([128, HL, T], BF16)
            kT = apool.tile([128, HL, T], BF16)
            Vh = apool.tile([128, NTT, HL, 66], BF16)

            nc.gpsimd.memset(qT[:], 0.0)
            nc.gpsimd.memset(kT[:], 0.0)
            nc.scalar.memzero(Vh[:, :, :, 64:66])
            nc.gpsimd.memset(Vh[:, :, :, 64:65], 1.0)
            if nqkv == 9:
                nc.gpsimd.memset(yT[:, 8, :], 0.0)
                nc.gpsimd.memset(yT[0:1, 8, :], 1.0)

            with ExitStack() as qkv_scope:
                psA = qkv_scope.enter_context(
                    tc.tile_pool(name="psA", bufs=2, space="PSUM"))

                # ---- LN1 + transpose to feature-major yT ----------------
                for t in range(NTT):
                    xt = work.tile([128, C], F32, tag="xt")
                    nc.sync.dma_start(xt[:], x_d[ts(t, 128), :])
                    bst = stats.tile([128, 2, 6], F32, tag="bst")
                    for sg in range(2):
                        nc.vector.bn_stats(bst[:, sg, :], xt[:, ts(sg, 512)])
                    mv = stats.tile([128, 2], F32, tag="mv")
                    nc.vector.bn_aggr(mv[:], bst[:])
                    std = stats.tile([128, 1], F32, tag="std")
                    nc.scalar.activation(std[:], mv[:, 1:2],
                                         mybir.ActivationFunctionType.Sqrt,
                                         bias=eps_ap[:])
                    rstd = stats.tile([128, 1], F32, tag="rstd")
                    nc.vector.reciprocal(rstd[:], std[:])
                    yb = work.tile([128, C], BF16, tag="yb")
                    nc.vector.tensor_scalar(yb[:], xt[:], mv[:, 0:1], rstd[:],
                                            mybir.AluOpType.subtract,
                                            mybir.AluOpType.mult)
                    for half in range(2):
                        pt = psA.tile([128, 4, 128], BF16, tag="tr")
                        for i in range(4):
                            cc = half * 4 + i
                            nc.tensor.transpose(pt[:, i, :],
                                                yb[:, ts(cc, 128)], ident[:])
                        nc.vector.tensor_copy(
                            yT[:, half * 4:(half + 1) * 4, ts(t, 128)], pt[:])

                # ---- QKV ------------------------------------------------
                for p in range(2):      # head pairs -> q/k feature-major
                    for blk in range(NTB):
                        for wsb, dstT in ((wq_sb, qT), (wk_sb, kT)):
                            ps = psA.tile([128, 512], F32, tag="qkv")
                            for cc in range(nqkv):
                                nc.tensor.matmul(ps[:], wsb[:, p, cc, :],
                                                 yT[:, cc, ts(blk, 512)],
                                                 start=(cc == 0),
                                                 stop=(cc == nqkv - 1))
                            nc.vector.tensor_copy(
                                dstT[0:64, 2 * p, ts(blk, 512)], ps[0:64, :])
                            nc.vector.tensor_copy(
                                dstT[0:64, 2 * p + 1, ts(blk, 512)],
                                ps[64:128, :])
                for t in range(NTT):    # V token-major
                    ps = psA.tile([128, 256], F32, tag="qkv")
                    for cc in range(nqkv):
                        nc.tensor.matmul(ps[:], yT[:, cc, ts(t, 128)],
                                         wv_sb[:, cc, :],
                                         start=(cc == 0), stop=(cc == nqkv - 1))
                    nc.scalar.copy(Vh[:, t, :, 0:64],
                                   ps.rearrange("p (h d) -> p h d", d=64))

            # ---- attention + per-block proj + pipelined RS --------------
            with ExitStack() as ps_scope:
                psS = ps_scope.enter_context(
                    tc.tile_pool(name="psS", bufs=3, space="PSUM"))
                psO = ps_scope.enter_context(
                    tc.tile_pool(name="psO", bufs=3, space="PSUM"))
                psJ = ps_scope.enter_context(
                    tc.tile_pool(name="psJ", bufs=2, space="PSUM"))
                dpad = apool.tile([128, 512], F32, tag="dpad")
                nc.gpsimd.memset(dpad[:], 0.0)
                ones64 = apool.tile([128, 64], F32, tag="ones64")
                nc.gpsimd.memset(ones64[:], 1.0)
                for c in range(NTB):
                    OnT_c = OnTp.tile([128, 2, 512], BF16, tag="OnT")
                    for h in range(HL):
                        po = psO.tile([128, 512], F32, tag="o")
                        t_hi = 4 * c + 4

                        def emit_s(t):
                            st = psS.tile([128, 512], F32, tag="s",
                                          name=f"s_{c}_{h}_{t}")
                            nc.tensor.matmul(st[:], kT[:, h, ts(t, 128)],
                                             qT[:, h, ts(c, 512)],
                                             start=True, stop=True)
                            pT = pipe.tile([128, 512], BF16, tag="pT",
                                           name=f"pT_{c}_{h}_{t}")
                            idx = h * 16 + t
                            nc.scalar.activation(
                                pT[:], st[:], mybir.ActivationFunctionType.Exp,
                                bias=ebias[:, idx:idx + 1])
                            if t >= 4 * c:
                                nc.vector.tensor_tensor(
                                    pT[:], pT[:], masks[:, t - 4 * c, :],
                                    mybir.AluOpType.mult)
                            return pT

                        pTs = {0: emit_s(0)}
                        if t_hi > 1:
                            pTs[1] = emit_s(1)
                        for t in range(t_hi):
                            nc.tensor.matmul(po[0:66, :], Vh[:, t, h, 0:66],
                                             pTs.pop(t)[:], start=(t == 0),
                                             stop=(t == t_hi - 1))
                            if t + 2 < t_hi:
                                pTs[t + 2] = emit_s(t + 2)
                        # denominator broadcast via f32r ones-matmul
                        nc.vector.tensor_copy(dpad[0:1, :], po[64:65, :])
                        otmp = pipe.tile([64, 512], F32, tag="otmp")
                        nc.vector.tensor_copy(otmp[:], po[0:64, :])
                        rb = psJ.tile([64, 512], F32, tag="pj")
                        nc.tensor.matmul(rb[:], ones64[:], dpad[:],
                                         start=True, stop=True)
                        rec = pipe.tile([64, 512], F32, tag="rec")
                        nc.vector.reciprocal(rec[:], rb[:])
                        nc.vector.tensor_tensor(
                            OnT_c[(h % 2) * 64:(h % 2) * 64 + 64, h // 2, :],
                            otmp[:], rec[:], mybir.AluOpType.mult)
                    # proj partial for this token block -> reduce-scatter
                    for i2 in range(4):
                        for nb in range(2):
                            pp_ = psJ.tile([128, 512], F32, tag="pj")
                            for cc in range(2):
                                nc.tensor.matmul(pp_[:],
                                                 OnT_c[:, cc, ts(i2, 128)],
                                                 wp_sb[:, cc, ts(nb, 512)],
                                                 start=(cc == 0), stop=(cc == 1))
                            pjs = pipe.tile([128, 512], BF16, tag="pjs")
                            nc.scalar.copy(pjs[:], pp_[:])
                            nc.sync.dma_start(
                                rs_ins[c][ts(i2, 128), ts(nb, 512)], pjs[:])
                    nc.gpsimd.collective_compute(
                        "ReduceScatter", mybir.AluOpType.add,
                        replica_groups=[[0, 1, 2, 3], [4, 5, 6, 7]],
                        ins=[rs_ins[c].opt()], outs=[rs_outs[c].opt()],
                    )

        # ---- residual + LN2 + SwiGLU MLP (row-parallel) -------------
        with ExitStack() as mlp_scope:
            mpool = mlp_scope.enter_context(tc.tile_pool(name="mlp", bufs=1))
            wstream = mlp_scope.enter_context(tc.tile_pool(name="wstream", bufs=2))
            wdpool = mlp_scope.enter_context(tc.tile_pool(name="wdpool", bufs=2))
            psC = mlp_scope.enter_context(
                tc.tile_pool(name="psC", bufs=2, space="PSUM"))

            x2 = mpool.tile([128, RT, C], F32)
            y2T = mpool.tile([128, 8, ROWS], BF16)
            gu = mpool.tile([128, MCH, ROWS], BF16)

            for r in range(RT):
                rst = work.tile([128, C], BF16, tag="rst")
                nc.sync.dma_start(rst[:], rs_outs[r][:, :])
                xrt = work.tile([128, C], F32, tag="xrt")
                nc.sync.dma_start(xrt[:], xr_d[ts(r, 128), :])
                nc.vector.tensor_tensor(x2[:, r, :], rst[:], xrt[:],
                                        mybir.AluOpType.add)
                bst = stats.tile([128, 2, 6], F32, tag="bst")
                for sg in range(2):
                    nc.vector.bn_stats(bst[:, sg, :], x2[:, r, ts(sg, 512)])
                mv = stats.tile([128, 2], F32, tag="mv")
                nc.vector.bn_aggr(mv[:], bst[:])
                std = stats.tile([128, 1], F32, tag="std")
                nc.scalar.activation(std[:], mv[:, 1:2],
                                     mybir.ActivationFunctionType.Sqrt,
                                     bias=eps_ap[:])
                rstd = stats.tile([128, 1], F32, tag="rstd")
                nc.vector.reciprocal(rstd[:], std[:])
                yb = work.tile([128, C], BF16, tag="yb")
                nc.vector.tensor_scalar(yb[:], x2[:, r, :], mv[:, 0:1], rstd[:],
                                        mybir.AluOpType.subtract,
                                        mybir.AluOpType.mult)
                for half in range(2):
                    pt = psC.tile([128, 4, 128], BF16, tag="tr2")
                    for i in range(4):
                        cc = half * 4 + i
                        nc.tensor.transpose(pt[:, i, :], yb[:, ts(cc, 128)],
                                            ident[:])
                    nc.vector.tensor_copy(
                        y2T[:, half * 4:(half + 1) * 4, ts(r, 128)], pt[:])

            for hc in range(MCH):
                wgt = wstream.tile([128, 8, 128], BF16, tag="wgt")
                nc.sync.dma_start(wgt[:], wg_d[:, :, ts(hc, 128)])
                wut = wstream.tile([128, 8, 128], BF16, tag="wut")
                nc.sync.dma_start(wut[:], wu_d[:, :, ts(hc, 128)])
                pg = psC.tile([128, 512], F32, tag="g")
                pu = psC.tile([128, 512], F32, tag="u")
                for cc in range(CCH):
                    nc.tensor.matmul(pg[:], wgt[:, cc, :], y2T[:, cc, :],
                                     start=(cc == 0), stop=(cc == CCH - 1))
                for cc in range(CCH):
                    nc.tensor.matmul(pu[:], wut[:, cc, :], y2T[:, cc, :],
                                     start=(cc == 0), stop=(cc == CCH - 1))
                gs = work.tile([128, 512], BF16, tag="gs")
                nc.scalar.activation(gs[:], pg[:],
                                     mybir.ActivationFunctionType.Silu,
                                     bias=bg_sb[:, hc:hc + 1])
                us = work.tile([128, 512], BF16, tag="us")
                nc.scalar.activation(us[:], pu[:],
                                     mybir.ActivationFunctionType.Identity,
                                     bias=bu_sb[:, hc:hc + 1])
                nc.vector.tensor_tensor(gu[:, hc, :], gs[:], us[:],
                                        mybir.AluOpType.mult)

            for nb in range(2):
                wdt = wdpool.tile([128, MCH, 512], BF16, tag="wdt")
                nc.sync.dma_start(wdt[:], wd_d[:, :, ts(nb, 512)])
                for tt in range(RT):
                    pd = psC.tile([128, 512], F32, tag="d")
                    for hc in range(MCH):
                        nc.tensor.matmul(pd[:], gu[:, hc, ts(tt, 128)],
                                         wdt[:, hc, :],
                                         start=(hc == 0), stop=(hc == MCH - 1))
                    o1 = work.tile([128, 512], F32, tag="o1")
                    nc.vector.tensor_tensor(o1[:], pd[:],
                                            x2[:, tt, ts(nb, 512)],
                                            mybir.AluOpType.add)
                    nc.vector.tensor_tensor(o1[:], o1[:],
                                            db_sb[:, ts(nb, 512)],
                                            mybir.AluOpType.add)
                    nc.sync.dma_start(out_d[ts(tt, 128), ts(nb, 512)], o1[:])

    _split_excess_waits(nc)
    return nc


# ----------------------------------------------------------------------------
# host-side input prep + launch
# ----------------------------------------------------------------------------
_cache = {}

def _get_nc(nqkv):
    if nqkv not in _cache:
        _cache[nqkv] = _build(nqkv)
    return _cache[nqkv]


def kernel(x, ln1_g, ln1_b, qkv_w, qkv_b, proj_w, proj_b,
           ln2_g, ln2_b, gate_w, gate_b, up_w, up_b, down_w, down_b):
    x = np.asarray(x, np.float32)
    f = lambda a: np.asarray(a, np.float32)
    ln1_g, ln1_b, qkv_b, proj_b, ln2_g, ln2_b = map(f, (
        ln1_g, ln1_b, qkv_b, proj_b, ln2_g, ln2_b))
    qkv_w, proj_w, gate_w, gate_b, up_w, up_b, down_w, down_b = map(f, (
        qkv_w, proj_w, gate_w, gate_b, up_w, up_b, down_w, down_b))

    slopes = _slopes()

    # fold LN affines into the consuming matmuls
    w1 = qkv_w * ln1_g[:, None]
    b1 = ln1_b @ qkv_w + qkv_b              # [3C]
    wg_f = gate_w * ln2_g[:, None]
    bg_f = ln2_b @ gate_w + gate_b          # [HID]
    wu_f = up_w * ln2_g[:, None]
    bu_f = ln2_b @ up_w + up_b

    nqkv = 9 if np.any(b1 != 0.0) else 8

    def qkv_aug(wcols, bcols, scale):
        # [C, 256] + bias row -> device layout [128, 2, 9, 128]
        wa = np.zeros((1152, 256), np.float32)
        wa[:C] = wcols * scale
        wa[C] = bcols * scale
        dev = np.zeros((128, 2, 9, 128), bf16)
        for p in range(2):
            blk = wa[:, p * 128:(p + 1) * 128]            # [1152, 128]
            dev[:, p] = blk.reshape(9, 128, 128).transpose(1, 0, 2).astype(bf16)
        return dev

    # masks (S^T diagonal tiles) and per-head exp biases
    pp = np.arange(128)[:, None]
    jj = np.arange(512)[None, :]
    masks_np = np.zeros((128, 4, 512), bf16)
    for v in range(4):
        masks_np[:, v, :] = (jj >= pp + 128 * v).astype(bf16)

    wgp = np.zeros((C, HIDP), np.float32); wgp[:, :HID] = wg_f
    wup = np.zeros((C, HIDP), np.float32); wup[:, :HID] = wu_f
    wdp = np.zeros((HIDP, 1024), np.float32); wdp[:HID] = down_w
    bgp = np.zeros(HIDP, np.float32); bgp[:HID] = bg_f
    bup = np.zeros(HIDP, np.float32); bup[:HID] = bu_f

    wg_dev = wgp.reshape(8, 128, HIDP).transpose(1, 0, 2).astype(bf16)
    wu_dev = wup.reshape(8, 128, HIDP).transpose(1, 0, 2).astype(bf16)
    wd_dev = wdp.reshape(MCH, 128, 1024).transpose(1, 0, 2).astype(bf16)
    bg_dev = bgp.reshape(MCH, 128).T.copy()
    bu_dev = bup.reshape(MCH, 128).T.copy()
    db_dev = np.broadcast_to(down_b, (128, 1024)).copy()

    in_maps = []
    for c in range(N_CORES):
        b, g = c // TP, c % TP
        heads = range(4 * g, 4 * g + 4)
        qcols = np.concatenate([np.arange(h * D, (h + 1) * D) for h in heads])
        kcols = qcols + C
        vcols = qcols + 2 * C

        wq_dev = qkv_aug(w1[:, qcols], b1[qcols], 0.125)
        wk_dev = qkv_aug(w1[:, kcols], b1[kcols], 1.0)
        wv_a = np.zeros((1152, 256), np.float32)
        wv_a[:C] = w1[:, vcols]
        wv_a[C] = b1[vcols]
        wv_dev = wv_a.reshape(9, 128, 256).transpose(1, 0, 2).astype(bf16)

        wp_rows = proj_w[qcols, :]                        # [256, 1024]
        wp_dev = wp_rows.reshape(2, 128, 1024).transpose(1, 0, 2).astype(bf16)

        eb = np.zeros((128, HL * 16), np.float32)
        for hl, h in enumerate(heads):
            sl = slopes[h]
            for t in range(16):
                eb[:, hl * 16 + t] = (-sl * (128 * t + np.arange(128))
                                      - CBIAS).astype(np.float32)

        in_maps.append({
            "x_full": x[b],
            "x_rows": np.concatenate(
                [x[b, 512 * r + 128 * g:512 * r + 128 * g + 128]
                 for r in range(4)], axis=0) + proj_b[None, :],
            "wq": wq_dev, "wk": wk_dev, "wv": wv_dev, "wproj": wp_dev,
            "wg": wg_dev, "wu": wu_dev, "wd": wd_dev,
            "bg": bg_dev, "bu": bu_dev, "dbias": db_dev,
            "masks": masks_np, "ebias": eb,
        })

    nc = _get_nc(nqkv)
    res = bass_utils.run_bass_kernel_spmd(
        nc, in_maps, core_ids=list(range(N_CORES)))

    out = np.empty((B, T, C), np.float32)
    for c in range(N_CORES):
        b, g = c // TP, c % TP
        orr = res.results[c]["out_rows"]
        for r in range(4):
            out[b, 512 * r + 128 * g:512 * r + 128 * g + 128] = \
                orr[128 * r:128 * (r + 1)]
    return out


# revision 15
# speedup vs baseline: 1.3113x; 1.0370x over previous
"""Trainium2 Bass kernel for EnhancedTransformerBlock (B=2,T=2048,C=1024,H=16,
SwiGLU HIDDEN=2730, ALiBi-abs + causal attention).

Sharding over 8 cores: batch (2) x head-groups (4 heads/core). Attention is
computed head-parallel with transposed scores S^T[tk,tq] so ALiBi + max-shift
collapse into a per-partition exp bias; softmax denominator rides as a ones
column in V through the PV matmul. A ReduceScatter over each batch group
converts the proj partials to row-sharded activations; the SwiGLU MLP then
runs row-parallel with full weights. Host reassembles row blocks.
"""
import sys, types
sys.path.insert(0, "/opt/trn_rl_repo")
import numpy as np
import ml_dtypes

import concourse.bass as bass
import concourse.tile as tile
from concourse import mybir
import concourse.bass_utils as bass_utils
import bass_rust
from concourse.masks import make_identity

# ----------------------------------------------------------------------------
# environment patches (walrus in this container accepts only 1 sync-wait/inst)
# ----------------------------------------------------------------------------
_DRAIN_WAIT_LIMIT = 1

def _patched_drain_and_barrier(self, tick_clock, wait_clock):
    nc = self.nc
    drain_inst = nc.sync.drain()
    wait_clock.add_sem_waits(
        drain_inst.ins, bass_rust.ScopedClock({None: tick_clock.global_clock})
    )
    si = drain_inst.ins.sync_info
    waits = list(si.on_wait) if si is not None else []
    if len(waits) > _DRAIN_WAIT_LIMIT:
        si.on_wait = waits[:_DRAIN_WAIT_LIMIT]
        for i in range(_DRAIN_WAIT_LIMIT, len(waits), _DRAIN_WAIT_LIMIT):
            d2 = nc.sync.drain()
            d2.ins.sync_info = bass_rust.SyncInfo(
                on_wait=waits[i:i + _DRAIN_WAIT_LIMIT], on_update=[]
            )
    nc.all_engine_barrier()
    popped = nc._tile_sem_poison_stack.pop()
    assert popped is self._sem_poison
    nc.clear_and_free_semaphores(list(self.sems.allocated().values()))
    nc.all_engine_barrier()


def _split_excess_waits(nc, limit=_DRAIN_WAIT_LIMIT):
    n = [0]
    for bb in nc.main_func.blocks:
        insts = bb.instructions
        out = []
        changed = False
        for inst in insts:
            si = inst.sync_info
            waits = list(si.on_wait) if si is not None else []
            if len(waits) > limit:
                changed = True
                keep = waits[-limit:]
                rest = waits[:-limit]
                for i in range(0, len(rest), limit):
                    n[0] += 1
                    d = mybir.InstNoOp(
                        name=f"waitsplit-{n[0]}", engine=inst.engine, ins=[], outs=[]
                    )
                    d.sync_info = bass_rust.SyncInfo(
                        on_wait=rest[i:i + limit], on_update=[]
                    )
                    out.append(d)
                si.on_wait = keep
            out.append(inst)
        if changed:
            bb.instructions = out


def _install_patches():
    tile.TileContext._drain_and_barrier = _patched_drain_and_barrier
    if "antenv.axon_hooks" not in sys.modules:
        try:
            from trn_agent_boot.trn_boot import _ntff_profile_via_ctypes
            hook = _ntff_profile_via_ctypes("/opt/axon/libaxon_pjrt.so")
        except Exception:
            hook = None
        mod = types.ModuleType("antenv.axon_hooks")
        mod.get_axon_ntff_profile_hook = lambda: hook
        mod.set_axon_ntff_profile_hook = lambda h: None
        sys.modules["antenv.axon_hooks"] = mod
        bass_utils.upload_artifacts = lambda tmpdir: tmpdir

_install_patches()

# ----------------------------------------------------------------------------
# problem constants
# ----------------------------------------------------------------------------
B, T, C = 2, 2048, 1024
H, D = 16, 64
HID, HIDP = 2730, 2816
N_CORES, TP = 8, 4
ROWS = T // TP          # 512 rows per core after reduce-scatter
HL = 4                  # local heads per core
EPS = 1e-5
CBIAS = 24.0
NEG = -1e30
F32, BF16 = mybir.dt.float32, mybir.dt.bfloat16
F32R = mybir.dt.float32r
bf16 = ml_dtypes.bfloat16
ts = bass.ts

NTT = T // 128          # 16 token tiles
NTB = T // 512          # 4 token blocks
CCH = C // 128          # 8 feature chunks
MCH = HIDP // 128       # 22 hidden chunks
RT = ROWS // 128        # 4 row tiles per core


def _slopes():
    i = np.arange(1, H + 1, dtype=np.float64)
    return (1.0 / np.power(2.0, 8.0 * i / H)).astype(np.float64)


# ----------------------------------------------------------------------------
# device program (identical on all 8 cores; per-core behavior via input data)
# ----------------------------------------------------------------------------
def _build(nqkv):
    nc = bass.Bass("TRN2", num_devices=N_CORES)

    x_d = nc.dram_tensor("x_full", [T, C], F32, kind="ExternalInput")
    xr_d = nc.dram_tensor("x_rows", [ROWS, C], F32, kind="ExternalInput")
    wq_d = nc.dram_tensor("wq", [128, 2, 9, 128], BF16, kind="ExternalInput")
    wk_d = nc.dram_tensor("wk", [128, 2, 9, 128], BF16, kind="ExternalInput")
    wv_d = nc.dram_tensor("wv", [128, 9, 256], BF16, kind="ExternalInput")
    wp_d = nc.dram_tensor("wproj", [128, 2, 1024], BF16, kind="ExternalInput")
    wg_d = nc.dram_tensor("wg", [128, 8, HIDP], BF16, kind="ExternalInput")
    wu_d = nc.dram_tensor("wu", [128, 8, HIDP], BF16, kind="ExternalInput")
    wd_d = nc.dram_tensor("wd", [128, MCH, 1024], BF16, kind="ExternalInput")
    bg_d = nc.dram_tensor("bg", [128, MCH], F32, kind="ExternalInput")
    bu_d = nc.dram_tensor("bu", [128, MCH], F32, kind="ExternalInput")
    db_d = nc.dram_tensor("dbias", [128, 1024], F32, kind="ExternalInput")
    mk_d = nc.dram_tensor("masks", [128, 4, 512], BF16, kind="ExternalInput")
    eb_d = nc.dram_tensor("ebias", [128, HL * 16], F32, kind="ExternalInput")

    out_d = nc.dram_tensor("out_rows", [ROWS, C], F32, kind="ExternalOutput")
    scr_d = nc.dram_tensor("scr", [16, 512], F32, kind="ExternalOutput")

    from contextlib import ExitStack
    with tile.TileContext(nc) as tc, ExitStack() as top:
        consts = top.enter_context(tc.tile_pool(name="consts", bufs=1))
        stats = top.enter_context(tc.tile_pool(name="stats", bufs=16))
        work = top.enter_context(tc.tile_pool(name="work", bufs=2))
        dramp = top.enter_context(tc.tile_pool(name="dram", bufs=1, space="DRAM"))

        ident = consts.tile([128, 128], BF16)
        make_identity(nc, ident[:])
        eps_ap = consts.tile([128, 1], F32)
        nc.vector.memset(eps_ap[:], EPS)
        masks = consts.tile([128, 4, 512], BF16)
        nc.gpsimd.dma_start(masks[:], mk_d[:, :, :])
        ebias = consts.tile([128, HL * 16], F32)
        nc.gpsimd.dma_start(ebias[:], eb_d[:, :])
        bg_sb = consts.tile([128, MCH], F32)
        nc.gpsimd.dma_start(bg_sb[:], bg_d[:, :])
        bu_sb = consts.tile([128, MCH], F32)
        nc.gpsimd.dma_start(bu_sb[:], bu_d[:, :])
        db_sb = consts.tile([128, 1024], F32)
        nc.gpsimd.dma_start(db_sb[:], db_d[:, :])

        rs_ins = [dramp.tile([512, C], BF16, name=f"rsin{i}") for i in range(NTB)]
        rs_outs = [dramp.tile([128, C], BF16, name=f"rsout{i}") for i in range(NTB)]

        with ExitStack() as attn_scope:
            apool = attn_scope.enter_context(tc.tile_pool(name="attn", bufs=1))
            pipe = attn_scope.enter_context(tc.tile_pool(name="pipe", bufs=4))
            OnTp = attn_scope.enter_context(tc.tile_pool(name="OnTp", bufs=2))

            wq_sb = apool.tile([128, 2, 9, 128], BF16)
            nc.gpsimd.dma_start(wq_sb[:], wq_d[:, :, :, :])
            wk_sb = apool.tile([128, 2, 9, 128], BF16)
            nc.gpsimd.dma_start(wk_sb[:], wk_d[:, :, :, :])
            wv_sb = apool.tile([128, 9, 256], BF16)
            nc.gpsimd.dma_start(wv_sb[:], wv_d[:, :, :])
            wp_sb = apool.tile([128, 2, 1024], BF16)
            nc.gpsimd.dma_start(wp_sb[:], wp_d[:, :, :])

            yT = apool.tile([128, nqkv, T], BF16)
            qT = apool.tile([128, HL, T], BF16)
            kT = apool.tile([128, HL, T], BF16)
            Vh = apool.tile([128, NTT, HL, 66], BF16)

            nc.gpsimd.memset(qT[:], 0.0)
            nc.gpsimd.memset(kT[:], 0.0)
            nc.scalar.memzero(Vh[:, :, :, 64:66])
            nc.gpsimd.memset(Vh[:, :, :, 64:65], 1.0)
            if nqkv == 9:
                nc.gpsimd.memset(yT[:, 8, :], 0.0)
                nc.gpsimd.memset(yT[0:1, 8, :], 1.0)

            with ExitStack() as qkv_scope:
                psA = qkv_scope.enter_context(
                    tc.tile_pool(name="psA", bufs=2, space="PSUM"))

                # ---- LN1 + transpose to feature-major yT ----------------
                for t in range(NTT):
                    xt = work.tile([128, C], F32, tag="xt")
                    nc.sync.dma_start(xt[:], x_d[ts(t, 128), :])
                    bst = stats.tile([128, 2, 6], F32, tag="bst")
                    for sg in range(2):
                        nc.vector.bn_stats(bst[:, sg, :], xt[:, ts(sg, 512)])
                    mv = stats.tile([128, 2], F32, tag="mv")
                    nc.vector.bn_aggr(mv[:], bst[:])
                    std = stats.tile([128, 1], F32, tag="std")
                    nc.scalar.activation(std[:], mv[:, 1:2],
                                         mybir.ActivationFunctionType.Sqrt,
                                         bias=eps_ap[:])
                    rstd = stats.tile([128, 1], F32, tag="rstd")
                    nc.vector.reciprocal(rstd[:], std[:])
                    yb = work.tile([128, C], BF16, tag="yb")
                    nc.vector.tensor_scalar(yb[:], xt[:], mv[:, 0:1], rstd[:],
                                            mybir.AluOpType.subtract,
                                            mybir.AluOpType.mult)
                    for half in range(2):
                        pt = psA.tile([128, 4, 128], BF16, tag="tr")
                        for i in range(4):
                            cc = half * 4 + i
                            nc.tensor.transpose(pt[:, i, :],
                                                yb[:, ts(cc, 128)], ident[:])
                        nc.vector.tensor_copy(
                            yT[:, half * 4:(half + 1) * 4, ts(t, 128)], pt[:])

                # ---- QKV ------------------------------------------------
                for p in range(2):      # head pairs -> q/k feature-major
                    for blk in range(NTB):
                        for wsb, dstT in ((wq_sb, qT), (wk_sb, kT)):
                            ps = psA.tile([128, 512], F32, tag="qkv")
                            for cc in range(nqkv):
                                nc.tensor.matmul(ps[:], wsb[:, p, cc, :],
                                                 yT[:, cc, ts(blk, 512)],
                                                 start=(cc == 0),
                                                 stop=(cc == nqkv - 1))
                            nc.vector.tensor_copy(
                                dstT[0:64, 2 * p, ts(blk, 512)], ps[0:64, :])
                            nc.vector.tensor_copy(
                                dstT[0:64, 2 * p + 1, ts(blk, 512)],
                                ps[64:128, :])
                for t in range(NTT):    # V token-major
                    ps = psA.tile([128, 256], F32, tag="qkv")
                    for cc in range(nqkv):
                        nc.tensor.matmul(ps[:], yT[:, cc, ts(t, 128)],
                                         wv_sb[:, cc, :],
                                         start=(cc == 0), stop=(cc == nqkv - 1))
                    nc.scalar.copy(Vh[:, t, :, 0:64],
                                   ps.rearrange("p (h d) -> p h d", d=64))

            # ---- attention + per-block proj + pipelined RS --------------
            with ExitStack() as ps_scope:
                psS = ps_scope.enter_context(
                    tc.tile_pool(name="psS", bufs=3, space="PSUM"))
                psO = ps_scope.enter_context(
                    tc.tile_pool(name="psO", bufs=3, space="PSUM"))
                psJ = ps_scope.enter_context(
                    tc.tile_pool(name="psJ", bufs=2, space="PSUM"))
                dpad = apool.tile([128, 512], F32, tag="dpad")
                nc.gpsimd.memset(dpad[:], 0.0)
                ones64 = apool.tile([128, 64], F32, tag="ones64")
                nc.gpsimd.memset(ones64[:], 1.0)
                for c in range(NTB):
                    OnT_c = OnTp.tile([128, 2, 512], BF16, tag="OnT")
                    for h in range(HL):
                        po = psO.tile([128, 512], F32, tag="o")
                        t_hi = 4 * c + 4

                        def emit_s(t):
                            st = psS.tile([128, 512], F32, tag="s",
                                          name=f"s_{c}_{h}_{t}")
                            nc.tensor.matmul(st[:], kT[:, h, ts(t, 128)],
                                             qT[:, h, ts(c, 512)],
                                             start=True, stop=True)
                            pT = pipe.tile([128, 512], BF16, tag="pT",
                                           name=f"pT_{c}_{h}_{t}")
                            idx = h * 16 + t
                            nc.scalar.activation(
                                pT[:], st[:], mybir.ActivationFunctionType.Exp,
                                bias=ebias[:, idx:idx + 1])
                            if t >= 4 * c:
                                nc.vector.tensor_tensor(
                                    pT[:], pT[:], masks[:, t - 4 * c, :],
                                    mybir.AluOpType.mult)
                            return pT

                        pTs = {0: emit_s(0)}
                        if t_hi > 1:
                            pTs[1] = emit_s(1)
                        for t in range(t_hi):
                            nc.tensor.matmul(po[0:66, :], Vh[:, t, h, 0:66],
                                             pTs.pop(t)[:], start=(t == 0),
                                             stop=(t == t_hi - 1))
                            if t + 2 < t_hi:
                                pTs[t + 2] = emit_s(t + 2)
                        # denominator broadcast via f32r ones-matmul
                        nc.vector.tensor_copy(dpad[0:1, :], po[64:65, :])
                        otmp = pipe.tile([64, 512], F32, tag="otmp")
                        nc.vector.tensor_copy(otmp[:], po[0:64, :])
                        rb = psJ.tile([64, 512], F32, tag="pj")
                        nc.tensor.matmul(rb[:], ones64[:], dpad[:],
                                         start=True, stop=True)
                        rec = pipe.tile([64, 512], F32, tag="rec")
                        nc.scalar.add_instruction(
                            mybir.InstActivation(
                                name=nc.get_next_instruction_name(),
                                func=mybir.ActivationFunctionType.Reciprocal,
                                ins=[nc.scalar.lower_ap(rb[:]),
                                     mybir.ImmediateValue(
                                         dtype=F32, value=0.0),
                                     mybir.ImmediateValue(
                                         dtype=F32, value=1.0),
                                     mybir.ImmediateValue(
                                         dtype=F32, value=0.0)],
                                outs=[nc.scalar.lower_ap(rec[:])],
                            ))
                        nc.vector.tensor_tensor(
                            OnT_c[(h % 2) * 64:(h % 2) * 64 + 64, h // 2, :],
                            otmp[:], rec[:], mybir.AluOpType.mult)
                    # proj partial for this token block -> reduce-scatter
                    for i2 in range(4):
                        for nb in range(2):
                            pp_ = psJ.tile([128, 512], F32, tag="pj")
                            for cc in range(2):
                                nc.tensor.matmul(pp_[:],
                                                 OnT_c[:, cc, ts(i2, 128)],
                                                 wp_sb[:, cc, ts(nb, 512)],
                                                 start=(cc == 0), stop=(cc == 1))
                            pjs = pipe.tile([128, 512], BF16, tag="pjs")
                            nc.scalar.copy(pjs[:], pp_[:])
                            nc.sync.dma_start(
                                rs_ins[c][ts(i2, 128), ts(nb, 512)], pjs[:])
                    nc.gpsimd.collective_compute(
                        "ReduceScatter", mybir.AluOpType.add,
                        replica_groups=[[0, 1, 2, 3], [4, 5, 6, 7]],
                        ins=[rs_ins[c].opt()], outs=[rs_outs[c].opt()],
                    )

        # ---- residual + LN2 + SwiGLU MLP (row-parallel) -------------
        with ExitStack() as mlp_scope:
            mpool = mlp_scope.enter_context(tc.tile_pool(name="mlp", bufs=1))
            wstream = mlp_scope.enter_context(tc.tile_pool(name="wstream", bufs=2))
            wdpool = mlp_scope.enter_context(tc.tile_pool(name="wdpool", bufs=2))
            psC = mlp_scope.enter_context(
                tc.tile_pool(name="psC", bufs=2, space="PSUM"))

            x2 = mpool.tile([128, RT, C], F32)
            y2T = mpool.tile([128, 8, ROWS], BF16)
            gu = mpool.tile([128, MCH, ROWS], BF16)

            for r in range(RT):
                rst = work.tile([128, C], BF16, tag="rst")
                nc.sync.dma_start(rst[:], rs_outs[r][:, :])
                xrt = work.tile([128, C], F32, tag="xrt")
                nc.sync.dma_start(xrt[:], xr_d[ts(r, 128), :])
                nc.vector.tensor_tensor(x2[:, r, :], rst[:], xrt[:],
                                        mybir.AluOpType.add)
                bst = stats.tile([128, 2, 6], F32, tag="bst")
                for sg in range(2):
                    nc.vector.bn_stats(bst[:, sg, :], x2[:, r, ts(sg, 512)])
                mv = stats.tile([128, 2], F32, tag="mv")
                nc.vector.bn_aggr(mv[:], bst[:])
                std = stats.tile([128, 1], F32, tag="std")
                nc.scalar.activation(std[:], mv[:, 1:2],
                                     mybir.ActivationFunctionType.Sqrt,
                                     bias=eps_ap[:])
                rstd = stats.tile([128, 1], F32, tag="rstd")
                nc.vector.reciprocal(rstd[:], std[:])
                yb = work.tile([128, C], BF16, tag="yb")
                nc.vector.tensor_scalar(yb[:], x2[:, r, :], mv[:, 0:1], rstd[:],
                                        mybir.AluOpType.subtract,
                                        mybir.AluOpType.mult)
                for half in range(2):
                    pt = psC.tile([128, 4, 128], BF16, tag="tr2")
                    for i in range(4):
                        cc = half * 4 + i
                        nc.tensor.transpose(pt[:, i, :], yb[:, ts(cc, 128)],
                                            ident[:])
                    nc.vector.tensor_copy(
                        y2T[:, half * 4:(half + 1) * 4, ts(r, 128)], pt[:])

            for hc in range(MCH):
                wgt = wstream.tile([128, 8, 128], BF16, tag="wgt")
                nc.sync.dma_start(wgt[:], wg_d[:, :, ts(hc, 128)])
                wut = wstream.tile([128, 8, 128], BF16, tag="wut")
                nc.sync.dma_start(wut[:], wu_d[:, :, ts(hc, 128)])
                pg = psC.tile([128, 512], F32, tag="g")
                pu = psC.tile([128, 512], F32, tag="u")
                for cc in range(CCH):
                    nc.tensor.matmul(pg[:], wgt[:, cc, :], y2T[:, cc, :],
                                     start=(cc == 0), stop=(cc == CCH - 1))
                for cc in range(CCH):
                    nc.tensor.matmul(pu[:], wut[:, cc, :], y2T[:, cc, :],
                                     start=(cc == 0), stop=(cc == CCH - 1))
                gs = work.tile([128, 512], BF16, tag="gs")
                nc.scalar.activation(gs[:], pg[:],
                                     mybir.ActivationFunctionType.Silu,
                                     bias=bg_sb[:, hc:hc + 1])
                us = work.tile([128, 512], BF16, tag="us")
                nc.scalar.activation(us[:], pu[:],
                                     mybir.ActivationFunctionType.Identity,
                                     bias=bu_sb[:, hc:hc + 1])
                nc.vector.tensor_tensor(gu[:, hc, :], gs[:], us[:],
                                        mybir.AluOpType.mult)

            for nb in range(2):
                wdt = wdpool.tile([128, MCH, 512], BF16, tag="wdt")
                nc.sync.dma_start(wdt[:], wd_d[:, :, ts(nb, 512)])
                for tt in range(RT):
                    pd = psC.tile([128, 512], F32, tag="d")
                    for hc in range(MCH):
                        nc.tensor.matmul(pd[:], gu[:, hc, ts(tt, 128)],
                                         wdt[:, hc, :],
                                         start=(hc == 0), stop=(hc == MCH - 1))
                    o1 = work.tile([128, 512], F32, tag="o1")
                    nc.vector.tensor_tensor(o1[:], pd[:],
                                            x2[:, tt, ts(nb, 512)],
                                            mybir.AluOpType.add)
                    nc.vector.tensor_tensor(o1[:], o1[:],
                                            db_sb[:, ts(nb, 512)],
                                            mybir.AluOpType.add)
                    nc.sync.dma_start(out_d[ts(tt, 128), ts(nb, 512)], o1[:])

    _split_excess_waits(nc)
    return nc


# ----------------------------------------------------------------------------
# host-side input prep + launch
# ----------------------------------------------------------------------------
_cache = {}

def _get_nc(nqkv):
    if nqkv not in _cache:
        _cache[nqkv] = _build(nqkv)
    return _cache[nqkv]


def kernel(x, ln1_g, ln1_b, qkv_w, qkv_b, proj_w, proj_b,
           ln2_g, ln2_b, gate_w, gate_b, up_w, up_b, down_w, down_b):
    x = np.asarray(x, np.float32)
    f = lambda a: np.asarray(a, np.float32)
    ln1_g, ln1_b, qkv_b, proj_b, ln2_g, ln2_b = map(f, (
        ln1_g, ln1_b, qkv_b, proj_b, ln2_g, ln2_b))
    qkv_w, proj_w, gate_w, gate_b, up_w, up_b, down_w, down_b = map(f, (
        qkv_w, proj_w, gate_w, gate_b, up_w, up_b, down_w, down_b))

    slopes = _slopes()

    # fold LN affines into the consuming matmuls
    w1 = qkv_w * ln1_g[:, None]
    b1 = ln1_b @ qkv_w + qkv_b              # [3C]
    wg_f = gate_w * ln2_g[:, None]
    bg_f = ln2_b @ gate_w + gate_b          # [HID]
    wu_f = up_w * ln2_g[:, None]
    bu_f = ln2_b @ up_w + up_b

    nqkv = 9 if np.any(b1 != 0.0) else 8

    def qkv_aug(wcols, bcols, scale):
        # [C, 256] + bias row -> device layout [128, 2, 9, 128]
        wa = np.zeros((1152, 256), np.float32)
        wa[:C] = wcols * scale
        wa[C] = bcols * scale
        dev = np.zeros((128, 2, 9, 128), bf16)
        for p in range(2):
            blk = wa[:, p * 128:(p + 1) * 128]            # [1152, 128]
            dev[:, p] = blk.reshape(9, 128, 128).transpose(1, 0, 2).astype(bf16)
        return dev

    # masks (S^T diagonal tiles) and per-head exp biases
    pp = np.arange(128)[:, None]
    jj = np.arange(512)[None, :]
    masks_np = np.zeros((128, 4, 512), bf16)
    for v in range(4):
        masks_np[:, v, :] = (jj >= pp + 128 * v).astype(bf16)

    wgp = np.zeros((C, HIDP), np.float32); wgp[:, :HID] = wg_f
    wup = np.zeros((C, HIDP), np.float32); wup[:, :HID] = wu_f
    wdp = np.zeros((HIDP, 1024), np.float32); wdp[:HID] = down_w
    bgp = np.zeros(HIDP, np.float32); bgp[:HID] = bg_f
    bup = np.zeros(HIDP, np.float32); bup[:HID] = bu_f

    wg_dev = wgp.reshape(8, 128, HIDP).transpose(1, 0, 2).astype(bf16)
    wu_dev = wup.reshape(8, 128, HIDP).transpose(1, 0, 2).astype(bf16)
    wd_dev = wdp.reshape(MCH, 128, 1024).transpose(1, 0, 2).astype(bf16)
    bg_dev = bgp.reshape(MCH, 128).T.copy()
    bu_dev = bup.reshape(MCH, 128).T.copy()
    db_dev = np.broadcast_to(down_b, (128, 1024)).copy()

    in_maps = []
    for c in range(N_CORES):
        b, g = c // TP, c % TP
        heads = range(4 * g, 4 * g + 4)
        qcols = np.concatenate([np.arange(h * D, (h + 1) * D) for h in heads])
        kcols = qcols + C
        vcols = qcols + 2 * C

        wq_dev = qkv_aug(w1[:, qcols], b1[qcols], 0.125)
        wk_dev = qkv_aug(w1[:, kcols], b1[kcols], 1.0)
        wv_a = np.zeros((1152, 256), np.float32)
        wv_a[:C] = w1[:, vcols]
        wv_a[C] = b1[vcols]
        wv_dev = wv_a.reshape(9, 128, 256).transpose(1, 0, 2).astype(bf16)

        wp_rows = proj_w[qcols, :]                        # [256, 1024]
        wp_dev = wp_rows.reshape(2, 128, 1024).transpose(1, 0, 2).astype(bf16)

        eb = np.zeros((128, HL * 16), np.float32)
        for hl, h in enumerate(heads):
            sl = slopes[h]
            for t in range(16):
                eb[:, hl * 16 + t] = (-sl * (128 * t + np.arange(128))
                                      - CBIAS).astype(np.float32)

        in_maps.append({
            "x_full": x[b],
            "x_rows": np.concatenate(
                [x[b, 512 * r + 128 * g:512 * r + 128 * g + 128]
                 for r in range(4)], axis=0) + proj_b[None, :],
            "wq": wq_dev, "wk": wk_dev, "wv": wv_dev, "wproj": wp_dev,
            "wg": wg_dev, "wu": wu_dev, "wd": wd_dev,
            "bg": bg_dev, "bu": bu_dev, "dbias": db_dev,
            "masks": masks_np, "ebias": eb,
        })

    nc = _get_nc(nqkv)
    res = bass_utils.run_bass_kernel_spmd(
        nc, in_maps, core_ids=list(range(N_CORES)))

    out = np.empty((B, T, C), np.float32)
    for c in range(N_CORES):
        b, g = c // TP, c % TP
        orr = res.results[c]["out_rows"]
        for r in range(4):
            out[b, 512 * r + 128 * g:512 * r + 128 * g + 128] = \
                orr[128 * r:128 * (r + 1)]
    return out
